# revision 33
# baseline (speedup 1.0000x reference)
"""NTM cell kernel for Trainium2 (8 NeuronCores, batch-parallel).

Strategy (per core, 8 batches):
  - prev_memory slice is cast-loaded f32->bf16 into SBUF (row-major M16).
  - The on-chip transpose to T16 runs on the TENSOR engine (128x128 bf16
    transposes into PSUM, ~1 cyc/row) instead of the DMA xbar, freeing the
    DMA pool for the HBM load; PSUM->SBUF cast copies alternate between
    the vector and scalar engines, elementwise squares (T2) between
    gpsimd and vector.
  - All O(N*D) reductions run on the tensor engine:
      * content dots + sum-of-squares streams over T16 / T2 (stationary
        rides the FWL weight path at ~0.5 cyc/col)
      * read-vector contraction with the memory chunk-pair as the
        128-col stationary and [w_r, w_r*w_w] as a 4-wide moving; the
        two chunk-halves land on partition halves and are folded after
        the final output transpose.
  - new_memory is never materialized; its dot/norm/read contributions are
    expanded algebraically in terms of streams over the ORIGINAL memory.
  - Addressing chains (softmax/gate/shift/sharpen) run on DVE/ACT/GPSIMD
    in a [128 x 64] layout (n = p*64 + c), with per-group buffers so the
    two batch-groups' chains pipeline instead of serializing.
  - Only one ACT table set is used (exp/ln); sqrt/sigmoid/tanh/softplus
    are rewritten via exp/ln so no table reloads occur.
"""

import sys

sys.path.insert(0, "/opt/trn_rl_repo")

import numpy as np

import concourse.bass as bass
import concourse.tile as tile
from concourse import mybir

F32 = mybir.dt.float32
BF16 = mybir.dt.bfloat16
AF = mybir.ActivationFunctionType
OP = mybir.AluOpType

B, N, D, C, IN, S = 64, 8192, 64, 256, 128, 3
NCORES = 8
BL = B // NCORES          # batches per core
P = 128                   # partitions
CH = N // P               # 64 chunks per batch (n = p*64 + c)
NPAIR = CH // 2           # 32 transposed tiles per batch
EPS = 1e-8

# whead column map
KR0, KR1 = 0, 64
BR, GR = 64, 65
SR0, SR1 = 66, 69
GAMR = 69
KW0, KW1 = 70, 134
BW, GW = 134, 135
SW0, SW1 = 136, 139
GAMW = 139
E0, E1 = 140, 204
A0, A1 = 204, 268
NHEAD = 268

# scalar table rows (S8 cols -> SC rows -> BC blocks of 8)
Q_BET_W, Q_G_W, Q_OMG_W, Q_SW0, Q_SW1, Q_SW2, Q_GAM_W, Q_NK2_W = range(8)
Q_BET_R, Q_G_R, Q_OMG_R, Q_SR0, Q_SR1, Q_SR2, Q_GAM_R, Q_NK2_R = range(8, 16)
Q_AKR, Q_AA = 16, 17
NQ = 18

GRP = 4  # batches per pipeline group

# ---------------------------------------------------------------------------
# workaround: the deployed walrus accepts only ONE sem-wait per instruction.
# After TileContext exits, hoist extra waits onto injected single-wait nops
# (drains on the SP engine, ENGINE_NOPs elsewhere).
# ---------------------------------------------------------------------------
import concourse.tile as tile_mod


def _split_multi_waits(nc):
    for f in nc.m.functions:
        for b in f.blocks:
            insts = b.instructions
            i = 0
            while i < len(insts):
                ins = insts[i]
                si = getattr(ins, "sync_info", None)
                if si is None or len(si.on_wait) <= 1:
                    i += 1
                    continue
                waits = list(si.on_wait)
                ins.sync_info = mybir.SyncInfo(
                    on_wait=[waits[-1]], on_update=list(si.on_update)
                )
                eng = nc.engines[ins.engine]
                new_nops = []
                for w in waits[:-1]:
                    nop = eng.isa(
                        nc.isa.Opcode.NEURON_ISA_TPB_OPCODE_NOP, {}
                    ).ins
                    nop.sync_info = mybir.SyncInfo(on_wait=[w], on_update=[])
                    new_nops.append(nop)
                for nop in new_nops:
                    for bb2 in f.blocks:
                        try:
                            bb2.instructions.remove(nop)
                            break
                        except ValueError:
                            pass
                for k, nop in enumerate(new_nops):
                    insts.insert(i + k, nop)
                i += len(new_nops) + 1


_orig_exit = tile_mod.TileContext.__exit__


def _patched_exit(self, *a, **k):
    import os
    r = _orig_exit(self, *a, **k)
    if not os.environ.get("NTM_NO_WAITFIX"):
        _split_multi_waits(self.nc)
    return r


if not getattr(tile_mod.TileContext, "_waitfix_patched", False):
    tile_mod.TileContext.__exit__ = _patched_exit
    tile_mod.TileContext._waitfix_patched = True


# ---------------------------------------------------------------------------
# kernel body
# ---------------------------------------------------------------------------

def _build_module():
    nc = bass.Bass()

    mem = nc.dram_tensor("mem", [BL, N, D], F32, kind="ExternalInput")
    x_in = nc.dram_tensor("x", [BL, IN], F32, kind="ExternalInput")
    rv_in = nc.dram_tensor("rv", [BL, D], F32, kind="ExternalInput")
    prw_in = nc.dram_tensor("prw", [BL, N], F32, kind="ExternalInput")
    pww_in = nc.dram_tensor("pww", [BL, N], F32, kind="ExternalInput")
    wctrl = nc.dram_tensor("wctrl", [IN + D, C], F32, kind="ExternalInput")
    bctrl = nc.dram_tensor("bctrl", [C], F32, kind="ExternalInput")
    whead = nc.dram_tensor("whead", [C, NHEAD], F32, kind="ExternalInput")
    bhead = nc.dram_tensor("bhead", [NHEAD], F32, kind="ExternalInput")
    ident = nc.dram_tensor("ident", [128, 128], F32, kind="ExternalInput")
    onest = nc.dram_tensor("onest", [128, 128], F32, kind="ExternalInput")
    permu = nc.dram_tensor("permu", [128, 128], F32, kind="ExternalInput")
    permd = nc.dram_tensor("permd", [128, 128], F32, kind="ExternalInput")
    seldr = nc.dram_tensor("sel", [32, NQ * 128], F32, kind="ExternalInput")
    out_d = nc.dram_tensor("out", [BL, C + D], F32, kind="ExternalOutput")

    with tile.TileContext(nc) as tc:
        _emit(nc, tc, mem, x_in, rv_in, prw_in, pww_in, wctrl, bctrl, whead,
              bhead, ident, onest, permu, permd, seldr, out_d)
    return nc


def _emit(nc, tc, mem, x_in, rv_in, prw_in, pww_in, wctrl, bctrl, whead,
          bhead, ident, onest, permu, permd, seldr, out_d):
    from contextlib import ExitStack

    ctx = ExitStack()
    ctx.enter_context(nc.allow_low_precision(
        reason="bf16 chain intermediates; rel-err budget 2e-2"))
    big = ctx.enter_context(tc.tile_pool(name="big", bufs=1))
    cons = ctx.enter_context(tc.tile_pool(name="cons", bufs=1))
    work = ctx.enter_context(tc.tile_pool(name="work", bufs=1))
    t16p = ctx.enter_context(tc.tile_pool(name="t16p", bufs=3))
    t2p = ctx.enter_context(tc.tile_pool(name="t2p", bufs=2))
    qallp = ctx.enter_context(tc.tile_pool(name="qallp", bufs=2))
    ps_tr = ctx.enter_context(tc.tile_pool(name="ps_tr", bufs=2, space="PSUM"))
    ps_stream = ctx.enter_context(tc.tile_pool(name="ps_stream", bufs=2, space="PSUM"))
    ps_misc = ctx.enter_context(tc.tile_pool(name="ps_misc", bufs=3, space="PSUM"))
    ps_rvp = ctx.enter_context(tc.tile_pool(name="ps_rvp", bufs=1, space="PSUM"))

    # ---------------- big memory load: issue FIRST ----------------
    # batch 0 on sync so its descriptors hit the rings first; the rest
    # sequentially behind it on gpsimd (per-ring FIFO keeps completion
    # roughly batch-ordered).
    # bf16 const cast-loads go FIRST on the gpsimd SW queue (tiny), then
    # the 8 per-batch memory cast-loads (f32->bf16 in the DGE) behind them.
    m16s = [big.tile([P, CH, D], BF16, tag=f"m16_{b}", name=f"m16_{b}")
            for b in range(BL)]

    identb_sb = cons.tile([128, 128], BF16, tag="identb")
    nc.gpsimd.dma_start(out=identb_sb, in_=ident[:])
    wh0 = cons.tile([128, NHEAD], BF16, tag="wh0")
    nc.gpsimd.dma_start(out=wh0, in_=whead[0:128, :])
    wh1 = cons.tile([128, NHEAD], BF16, tag="wh1")
    nc.gpsimd.dma_start(out=wh1, in_=whead[128:256, :])
    bh_sb = cons.tile([1, NHEAD], BF16, tag="bh")
    nc.gpsimd.dma_start(out=bh_sb, in_=bhead.rearrange("(o n) -> o n", o=1))
    selb_sb = cons.tile([32, NQ * 128], BF16, tag="selb")
    nc.gpsimd.dma_start(out=selb_sb, in_=seldr[:])
    onesb_sb = cons.tile([1, 128], BF16, tag="onesb")
    nc.gpsimd.dma_start(out=onesb_sb, in_=onest[0:1, :])
    permub_sb = cons.tile([128, 128], BF16, tag="permub")
    nc.gpsimd.dma_start(out=permub_sb, in_=permu[:])
    permdb_sb = cons.tile([128, 128], BF16, tag="permdb")
    nc.gpsimd.dma_start(out=permdb_sb, in_=permd[:])
    for b in range(BL):
        nc.gpsimd.dma_start(
            out=m16s[b], in_=mem[b].rearrange("(p c) d -> p c d", p=128)
        )

    wc0 = cons.tile([128, C], F32, tag="wc0")
    nc.sync.dma_start(out=wc0, in_=wctrl[0:128, :])
    wc1 = cons.tile([64, C], F32, tag="wc1")
    nc.sync.dma_start(out=wc1, in_=wctrl[128:192, :])
    bc_sb = cons.tile([128, 2], F32, tag="bc")
    nc.sync.dma_start(out=bc_sb, in_=bctrl.rearrange("(j p) -> p j", p=128))
    xt_in = cons.tile([BL, IN], F32, tag="xt_in")
    nc.sync.dma_start(out=xt_in, in_=x_in[:])
    rv_sb = cons.tile([BL, D], F32, tag="rv_sb")
    nc.sync.dma_start(out=rv_sb, in_=rv_in[:])
    ident_sb = cons.tile([128, 128], F32, tag="ident")
    nc.sync.dma_start(out=ident_sb, in_=ident[:])

    ones_sb = cons.tile([128, 128], F32, tag="ones")
    nc.scalar.dma_start(out=ones_sb, in_=onest[:])
    permu_sb = cons.tile([128, 128], F32, tag="permu")
    nc.scalar.dma_start(out=permu_sb, in_=permu[:])
    permd_sb = cons.tile([128, 128], F32, tag="permd")
    nc.scalar.dma_start(out=permd_sb, in_=permd[:])
    pw_w = cons.tile([128, BL, CH], F32, tag="pw_w")
    nc.scalar.dma_start(out=pw_w, in_=pww_in.rearrange("b (p c) -> p b c", p=128))
    pw_r = cons.tile([128, BL, CH], F32, tag="pw_r")
    nc.scalar.dma_start(out=pw_r, in_=prw_in.rearrange("b (p c) -> p b c", p=128))


    # ---------------- controller: hT = relu(W_ctrl^T @ ctrl_in^T + b) -------
    ps_xt = ps_misc.tile([128, 144], F32, tag="pm")
    nc.tensor.transpose(ps_xt[:, 0:BL], xt_in, ident_sb[0:BL, 0:BL])
    xT = work.tile([128, BL], F32, tag="xT")
    nc.vector.tensor_copy(xT, ps_xt[:, 0:BL])
    ps_rt = ps_misc.tile([128, 144], F32, tag="pm")
    nc.tensor.transpose(ps_rt[0:D, 0:BL], rv_sb, ident_sb[0:BL, 0:BL])
    rvT = work.tile([64, BL], F32, tag="rvT")
    nc.vector.tensor_copy(rvT, ps_rt[0:D, 0:BL])

    hT_sb = []
    for j in range(2):
        ps_h = ps_misc.tile([128, 144], F32, tag="pm")
        nc.tensor.matmul(ps_h[:, 0:BL], wc0[:, j * 128:(j + 1) * 128], xT,
                         start=True, stop=False)
        nc.tensor.matmul(ps_h[:, 0:BL], wc1[:, j * 128:(j + 1) * 128], rvT,
                         start=False, stop=True)
        h_j = work.tile([128, BL], F32, tag=f"hT{j}")
        nc.scalar.activation(h_j, ps_h[:, 0:BL], AF.Relu,
                             bias=bc_sb[:, j:j + 1], scale=1.0)
        hT_sb.append(h_j)

    # ---------------- head params P = h @ Whead + bhead (bf16) ----------
    hT_b = []
    for j in range(2):
        hb = work.tile([128, BL], BF16, tag=f"hTb{j}", name=f"hTb{j}")
        nc.vector.tensor_copy(hb, hT_sb[j])
        hT_b.append(hb)
    ps_p = ps_misc.tile([BL, 512], F32, tag="pm")
    nc.tensor.matmul(ps_p[:, 0:NHEAD], hT_b[0], wh0, start=True, stop=False)
    nc.tensor.matmul(ps_p[:, 0:NHEAD], hT_b[1], wh1, start=False, stop=False)
    nc.tensor.matmul(ps_p[:, 0:NHEAD], onesb_sb[0:1, 0:BL], bh_sb,
                     start=False, stop=True)
    p_sb = work.tile([BL, NHEAD], F32, tag="p_sb")
    nc.vector.tensor_copy(p_sb, ps_p[:, 0:NHEAD])

    # ---------------- VA: per-batch d-vectors [BL, 8*64] ----------------
    # vec order: 0 k_w, 1 k_r, 2 e*k_r, 3 a, 4 a*e, 5 ones, 6 e, 7 e^2
    va = work.tile([BL, 512], F32, tag="va")
    nc.vector.tensor_copy(va[:, 0:64], p_sb[:, KW0:KW1])
    nc.vector.tensor_copy(va[:, 64:128], p_sb[:, KR0:KR1])

    def _sigmoid(dst, src):  # dst = 1/(1+exp(-src))
        nc.scalar.activation(dst, src, AF.Exp, scale=-1.0)
        nc.vector.tensor_scalar_add(dst, dst, 1.0)
        nc.vector.reciprocal(dst, dst)

    # e = sigmoid(P_e) -> va[:, 384:448]
    _sigmoid(va[:, 384:448], p_sb[:, E0:E1])
    # a = tanh(P_a) = 1 - 2/(exp(2x)+1) -> va[:, 192:256]
    nc.scalar.activation(va[:, 192:256], p_sb[:, A0:A1], AF.Exp, scale=2.0)
    nc.vector.tensor_scalar_add(va[:, 192:256], va[:, 192:256], 1.0)
    nc.vector.reciprocal(va[:, 192:256], va[:, 192:256])
    nc.vector.tensor_scalar(va[:, 192:256], va[:, 192:256], -2.0, 1.0,
                            op0=OP.mult, op1=OP.add)
    # beta' = softplus(P_beta) * rsqrt(||k||^2): folded into the stream
    # vectors so the chain's beta-mul and nk2-normalization disappear.
    tmp64p = work.tile([BL, 64], F32, tag="tmp64p")
    bp = work.tile([BL, 4], F32, tag="bp")
    nc.vector.tensor_mul(tmp64p, va[:, 0:64], va[:, 0:64])
    nc.vector.reduce_sum(bp[:, 0:1], tmp64p, axis=mybir.AxisListType.X)
    nc.vector.tensor_mul(tmp64p, va[:, 64:128], va[:, 64:128])
    nc.vector.reduce_sum(bp[:, 1:2], tmp64p, axis=mybir.AxisListType.X)
    nc.scalar.activation(bp[:, 0:2], bp[:, 0:2], AF.Ln)
    nc.scalar.activation(bp[:, 0:2], bp[:, 0:2], AF.Exp, scale=-0.5)
    nc.scalar.activation(bp[:, 2:3], p_sb[:, BW:BW + 1], AF.Exp)
    nc.scalar.activation(bp[:, 3:4], p_sb[:, BR:BR + 1], AF.Exp)
    nc.vector.tensor_scalar_add(bp[:, 2:4], bp[:, 2:4], 1.0)
    nc.scalar.activation(bp[:, 2:4], bp[:, 2:4], AF.Ln)
    nc.vector.tensor_mul(bp[:, 2:3], bp[:, 2:3], bp[:, 0:1])
    nc.vector.tensor_mul(bp[:, 3:4], bp[:, 3:4], bp[:, 1:2])
    nc.vector.tensor_scalar(va[:, 0:64], va[:, 0:64], bp[:, 2:3], None,
                            op0=OP.mult)
    nc.vector.tensor_scalar(va[:, 64:128], va[:, 64:128], bp[:, 3:4], None,
                            op0=OP.mult)
    # e*k_r (scaled), a*e, ones, e^2
    nc.vector.tensor_mul(va[:, 128:192], va[:, 384:448], va[:, 64:128])
    nc.vector.tensor_mul(va[:, 256:320], va[:, 192:256], va[:, 384:448])
    nc.vector.memset(va[:, 320:384], 1.0)
    nc.vector.tensor_mul(va[:, 448:512], va[:, 384:448], va[:, 384:448])

    # ---------------- VTD: transposed vectors with zero-halves --------------
    # VTD[p, half, vec, b]; half 0: rows 0-63 hold vec, rows 64-127 zero.
    vtd = work.tile([128, 2, 8, BL], BF16, tag="vtd")
    nc.vector.memset(vtd, 0.0)
    vapad = work.tile([BL, 8, 128], F32, tag="vapad")
    nc.vector.memset(vapad, 0.0)
    for v in range(8):
        nc.vector.tensor_copy(vapad[:, v, 64:128], va[:, v * 64:(v + 1) * 64])
    ps_top = ps_misc.tile([128, 144], F32, tag="pm")
    ps_bot = ps_misc.tile([128, 144], F32, tag="pm")
    for v in range(8):
        nc.tensor.transpose(ps_top[0:64, v * BL:(v + 1) * BL],
                            va[:, v * 64:(v + 1) * 64],
                            ident_sb[0:BL, 0:BL])
        nc.tensor.transpose(ps_bot[:, v * BL:(v + 1) * BL],
                            vapad[:, v, :], ident_sb[0:BL, 0:BL])
    nc.vector.tensor_copy(
        vtd[0:64].rearrange("p h v b -> p (h v b)")[:, 0:64],
        ps_top[0:64, 0:64])
    nc.vector.tensor_copy(
        vtd[64:128].rearrange("p h v b -> p (h v b)")[:, 64:128],
        ps_bot[64:128, 0:64])
    # e/a duplicated across both partition halves for the rv assembly:
    # ea_dup[b, 0:64] = e, [64:128] = e  (same for a) -> transpose -> [128, BL]
    ea_dup = work.tile([BL, 2, 128], F32, tag="ea_dup")
    nc.vector.tensor_copy(ea_dup[:, 0, 0:64], va[:, 384:448])
    nc.vector.tensor_copy(ea_dup[:, 0, 64:128], va[:, 384:448])
    nc.vector.tensor_copy(ea_dup[:, 1, 0:64], va[:, 192:256])
    nc.vector.tensor_copy(ea_dup[:, 1, 64:128], va[:, 192:256])
    ps_ea = ps_misc.tile([128, 144], F32, tag="pm")
    nc.tensor.transpose(ps_ea[:, 0:BL], ea_dup[:, 0, :], ident_sb[0:BL, 0:BL])
    nc.tensor.transpose(ps_ea[:, BL:2 * BL], ea_dup[:, 1, :],
                        ident_sb[0:BL, 0:BL])
    e2_sb = work.tile([128, BL], F32, tag="e2_sb")
    nc.vector.tensor_copy(e2_sb, ps_ea[:, 0:BL])
    a2_sb = work.tile([128, BL], F32, tag="a2_sb")
    nc.vector.tensor_copy(a2_sb, ps_ea[:, BL:2 * BL])

    # ---------------- per-batch scalars S8 [BL, 32] ----------------
    s8 = work.tile([BL, 32], F32, tag="s8")
    nc.vector.memset(s8, 0.0)
    tmp64 = work.tile([BL, 64], F32, tag="tmp64")

    def _softplus(dst, src):  # ln(1 + exp(src))
        nc.scalar.activation(dst, src, AF.Exp)
        nc.vector.tensor_scalar_add(dst, dst, 1.0)
        nc.scalar.activation(dst, dst, AF.Ln)

    def _softmax3(dst, src):
        ex3 = work.tile([BL, 3], F32, tag="ex3")
        nc.scalar.activation(ex3, src, AF.Exp)
        sm = work.tile([BL, 1], F32, tag="sm3")
        nc.vector.reduce_sum(sm, ex3, axis=mybir.AxisListType.X)
        nc.vector.reciprocal(sm, sm)
        nc.vector.tensor_scalar(dst, ex3, sm, None, op0=OP.mult)

    _softplus(s8[:, Q_BET_W:Q_BET_W + 1], p_sb[:, BW:BW + 1])
    _sigmoid(s8[:, Q_G_W:Q_G_W + 1], p_sb[:, GW:GW + 1])
    nc.vector.tensor_scalar(s8[:, Q_OMG_W:Q_OMG_W + 1],
                            s8[:, Q_G_W:Q_G_W + 1], -1.0, 1.0,
                            op0=OP.mult, op1=OP.add)
    _softmax3(s8[:, Q_SW0:Q_SW0 + 3], p_sb[:, SW0:SW1])
    _softplus(s8[:, Q_GAM_W:Q_GAM_W + 1], p_sb[:, GAMW:GAMW + 1])
    nc.vector.tensor_scalar_add(s8[:, Q_GAM_W:Q_GAM_W + 1],
                                s8[:, Q_GAM_W:Q_GAM_W + 1], 1.0)
    nc.vector.tensor_mul(tmp64, va[:, 0:64], va[:, 0:64])
    nc.vector.reduce_sum(s8[:, Q_NK2_W:Q_NK2_W + 1], tmp64,
                         axis=mybir.AxisListType.X)

    _softplus(s8[:, Q_BET_R:Q_BET_R + 1], p_sb[:, BR:BR + 1])
    _sigmoid(s8[:, Q_G_R:Q_G_R + 1], p_sb[:, GR:GR + 1])
    nc.vector.tensor_scalar(s8[:, Q_OMG_R:Q_OMG_R + 1],
                            s8[:, Q_G_R:Q_G_R + 1], -1.0, 1.0,
                            op0=OP.mult, op1=OP.add)
    _softmax3(s8[:, Q_SR0:Q_SR0 + 3], p_sb[:, SR0:SR1])
    _softplus(s8[:, Q_GAM_R:Q_GAM_R + 1], p_sb[:, GAMR:GAMR + 1])
    nc.vector.tensor_scalar_add(s8[:, Q_GAM_R:Q_GAM_R + 1],
                                s8[:, Q_GAM_R:Q_GAM_R + 1], 1.0)
    nc.vector.tensor_mul(tmp64, va[:, 64:128], va[:, 64:128])
    nc.vector.reduce_sum(s8[:, Q_NK2_R:Q_NK2_R + 1], tmp64,
                         axis=mybir.AxisListType.X)

    nc.vector.tensor_mul(tmp64, va[:, 192:256], va[:, 64:128])
    nc.vector.reduce_sum(s8[:, Q_AKR:Q_AKR + 1], tmp64,
                         axis=mybir.AxisListType.X)
    nc.vector.tensor_mul(tmp64, va[:, 192:256], va[:, 192:256])
    nc.vector.reduce_sum(s8[:, Q_AA:Q_AA + 1], tmp64,
                         axis=mybir.AxisListType.X)

    # transpose S8 -> SC [32, BL] and broadcast -> BC [128, NQ*8]
    ps_sc = ps_misc.tile([128, 144], F32, tag="pm")
    nc.tensor.transpose(ps_sc[0:32, 0:BL], s8, ident_sb[0:BL, 0:BL])
    sc_sb = work.tile([32, BL], BF16, tag="sc_sb")
    nc.vector.tensor_copy(sc_sb, ps_sc[0:32, 0:BL])
    ps_bc = ps_misc.tile([128, 144], F32, tag="pm")
    for q in range(NQ):
        nc.tensor.matmul(ps_bc[:, q * BL:(q + 1) * BL],
                         selb_sb[:, q * 128:(q + 1) * 128], sc_sb,
                         start=True, stop=True)
    bc_all = work.tile([128, NQ * BL], BF16, tag="bc_all")
    nc.vector.tensor_copy(bc_all, ps_bc[:, 0:NQ * BL])
    bc_f32 = work.tile([128, NQ * BL], F32, tag="bc_f32")
    nc.vector.tensor_copy(bc_f32, ps_bc[:, 0:NQ * BL])

    def BCF(q, b):
        return bc_f32[:, q * BL + b:q * BL + b + 1]

    def BC(q, b):
        return bc_all[:, q * BL + b:q * BL + b + 1]

    # ---------------- output staging ----------------
    out_sb = work.tile([BL, C + D], F32, tag="out_sb")
    ps_ho = ps_misc.tile([128, 144], F32, tag="pm")
    nc.tensor.transpose(ps_ho[0:BL, 0:128], hT_sb[0], ident_sb)
    nc.vector.tensor_copy(out_sb[:, 0:128], ps_ho[0:BL, 0:128])
    ps_ho2 = ps_misc.tile([128, 144], F32, tag="pm")
    nc.tensor.transpose(ps_ho2[0:BL, 0:128], hT_sb[1], ident_sb)
    nc.vector.tensor_copy(out_sb[:, 128:256], ps_ho2[0:BL, 0:128])

    swr_sb = work.tile([1, BL], F32, tag="swr_sb")
    r12_all = work.tile([128, BL, 2], F32, tag="r12_all")

    # ---------------- helpers for grouped heavy phase ----------------
    def scb3(q, gs):
        base = bc_all[:, q * BL + gs:q * BL + gs + GRP]
        return bass.AP(tensor=base.tensor, offset=base.offset,
                       ap=[base.ap[0], base.ap[1], [0, CH]])

    def scb3n(q, gs, n):
        base = bc_all[:, q * BL + gs:q * BL + gs + GRP]
        return bass.AP(tensor=base.tensor, offset=base.offset,
                       ap=[base.ap[0], base.ap[1], [0, n]])

    def bc3(t8):
        base = t8[:, :]
        return bass.AP(tensor=base.tensor, offset=base.offset,
                       ap=[base.ap[0], base.ap[1], [0, CH]])

    def ctile(tag, gi):
        tg = f"{tag}_g{gi}"
        return work.tile([P, GRP, CH], BF16, tag=tg, name=tg)

    def gtile(tag, gi, dt=F32):
        tg = f"{tag}_g{gi}"
        return work.tile([128, GRP], dt, tag=tg, name=tg)

    def psum_colsum_bcast(cs8, gi, eps=None, tag="tot"):
        # one matmul with a full ones stationary both sums over partitions
        # and broadcasts the per-batch total to every output partition
        ps_t = ps_misc.tile([128, 144], F32, tag="pm")
        nc.tensor.matmul(ps_t[:, 0:GRP], ones_sb, cs8, start=True, stop=True)
        rt = gtile(tag + "_rt", gi)
        if eps is not None:
            nc.vector.tensor_scalar_add(rt, ps_t[:, 0:GRP], eps)
            nc.vector.reciprocal(rt, rt)
        else:
            nc.vector.reciprocal(rt, ps_t[:, 0:GRP])
        return rt

    def w_chain_all(dk_v, ssm_v, pw_all, qo, gs, gi, dst):
        bet, g_, omg, s0, s1, s2, gam, nk2 = (qo + i for i in range(8))
        v = ctile("wc_v", gi)
        nc.scalar.activation(v, ssm_v, AF.Ln)
        inv = ctile("wc_inv", gi)
        nc.scalar.activation(inv, v, AF.Exp, scale=-0.5)
        bsim = ctile("wc_bsim", gi)
        nc.vector.tensor_mul(bsim, dk_v, inv)
        # exp + per-partition row-sum fused on ACT, one per batch
        ex = ctile("wc_ex", gi)
        cs = gtile("wc_cs", gi, F32)
        for j in range(GRP):
            nc.scalar.activation(ex[:, j], bsim[:, j], AF.Exp,
                                 accum_out=cs[:, j:j + 1])
        # sharpening is scale-invariant, so fold the content-softmax
        # denominator T into the interpolation instead of normalizing:
        # ws' = g*ex + T*(1-g)*pw  (T broadcast by the colsum matmul)
        ps_T = ps_misc.tile([128, 144], F32, tag="pm")
        nc.tensor.matmul(ps_T[:, 0:GRP], ones_sb, cs, start=True, stop=True)
        omgT = gtile("wc_omgT", gi)
        nc.vector.tensor_mul(omgT, ps_T[:, 0:GRP],
                             bc_all[:, omg * BL + gs:omg * BL + gs + GRP])
        t9 = ctile("wc_t9", gi)
        nc.vector.tensor_mul(t9, pw_all, bc3(omgT))
        wg = ctile("wc_wg", gi)
        for j in range(GRP):
            nc.scalar.activation(wg[:, j], ex[:, j], AF.Copy,
                                 scale=BCF(g_, gs + j))
        nc.vector.tensor_add(wg, wg, t9)
        # circular shift: body via shifted APs, boundary cols via perm matmuls
        ps_sh = ps_misc.tile([128, 144], F32, tag="pm")
        nc.tensor.matmul(ps_sh[:, 0:GRP], permub_sb, wg[:, :, 0],
                         start=True, stop=True)
        nc.tensor.matmul(ps_sh[:, GRP:2 * GRP], permdb_sb, wg[:, :, CH - 1],
                         start=True, stop=True)
        ws = ctile("wc_ws", gi)
        for j in range(GRP):
            nc.scalar.activation(ws[:, j], wg[:, j], AF.Copy,
                                 scale=BCF(s1, gs + j))
        tA = ctile("wc_tA", gi)
        nc.vector.tensor_mul(tA[:, :, 0:CH - 1], wg[:, :, 1:CH],
                             scb3n(s0, gs, CH - 1))
        nc.vector.tensor_add(ws[:, :, 0:CH - 1], ws[:, :, 0:CH - 1],
                             tA[:, :, 0:CH - 1])
        nc.vector.tensor_mul(tA[:, :, 1:CH], wg[:, :, 0:CH - 1],
                             scb3n(s2, gs, CH - 1))
        nc.vector.tensor_add(ws[:, :, 1:CH], ws[:, :, 1:CH],
                             tA[:, :, 1:CH])
        bnd = work.tile([128, 2 * GRP], F32, tag=f"wc_bnd_g{gi}",
                        name=f"wc_bnd_g{gi}")
        nc.vector.tensor_mul(bnd[:, 0:GRP], ps_sh[:, 0:GRP],
                             bc_all[:, s0 * BL + gs:s0 * BL + gs + GRP])
        nc.vector.tensor_mul(bnd[:, GRP:2 * GRP], ps_sh[:, GRP:2 * GRP],
                             bc_all[:, s2 * BL + gs:s2 * BL + gs + GRP])
        nc.vector.tensor_add(ws[:, :, CH - 1], ws[:, :, CH - 1],
                             bnd[:, 0:GRP])
        nc.vector.tensor_add(ws[:, :, 0], ws[:, :, 0], bnd[:, GRP:2 * GRP])
        # sharpening: wp = exp(gam * ln(ws)) with fused row-sums
        lg = ctile("wc_lg", gi)
        nc.scalar.activation(lg, ws, AF.Ln)
        wp = ctile("wc_wp", gi)
        cs2 = gtile("wc_cs2", gi, F32)
        for j in range(GRP):
            nc.scalar.activation(wp[:, j], lg[:, j], AF.Exp,
                                 scale=BCF(gam, gs + j),
                                 accum_out=cs2[:, j:j + 1])
        rt2 = psum_colsum_bcast(cs2, gi, eps=EPS, tag="wc_t2")
        for j in range(GRP):
            nc.scalar.activation(dst[:, j], wp[:, j], AF.Copy,
                                 scale=rt2[:, j:j + 1])

    # ---------------- per-batch heavy stream ----------------
    qalls = {}

    def emit_batch(b):
        gi, bb = b // GRP, b % GRP
        if bb == 0:
            qalls[gi] = qallp.tile([P, GRP, 512], BF16, tag="qall",
                                   name="qall")
        qall = qalls[gi]
        t16b = t16p.tile([P, NPAIR, 128], BF16, tag="t16b", name="t16b")
        t2b = t2p.tile([P, NPAIR, 128], BF16, tag="t2b", name="t2b")
        m16f = m16s[b].rearrange("p c d -> p (c d)")
        # transposes in quads sharing one PSUM bank; copies alternate
        # vector/scalar, squares alternate gpsimd/vector
        for q in range(4):
            ps_t = ps_tr.tile([128, 1024], BF16, tag="ps_t")
            for k in range(8):
                cp = q * 8 + k
                nc.tensor.transpose(ps_t[:, k * 128:(k + 1) * 128],
                                    m16f[:, cp * 128:(cp + 1) * 128],
                                    identb_sb)
            t16v = t16b[:, q * 8:(q + 1) * 8].rearrange("p a q -> p (a q)")
            t2v = t2b[:, q * 8:(q + 1) * 8].rearrange("p a q -> p (a q)")
            if q == 2:
                nc.vector.tensor_copy(t16v, ps_t)
            else:
                nc.scalar.activation(t16v, ps_t, AF.Copy)
            if q == 1:
                big2 = t16b[:, 0:16].rearrange("p a q -> p (a q)")
                big2d = t2b[:, 0:16].rearrange("p a q -> p (a q)")
                nc.gpsimd.tensor_mul(big2d, big2, big2)
            elif q >= 2:
                nc.vector.tensor_mul(t2v, t16v, t16v)
        # streams
        pb = ps_stream.tile([128, 512], F32, tag="pb")
        rhs_m = vtd[:, :, 0:5, b].rearrange("p h v -> p v h")
        rhs_s = vtd[:, :, 5:8, b].rearrange("p h v -> p v h")
        for cp in range(NPAIR):
            nc.tensor.matmul(pb[:, cp * 16:cp * 16 + 10],
                             t16b[:, cp], rhs_m, start=True, stop=True)
        for cp in range(NPAIR):
            nc.tensor.matmul(pb[:, cp * 16 + 10:cp * 16 + 16],
                             t2b[:, cp], rhs_s, start=True, stop=True)
        # de-interleave (cp, 2v+h) -> (v, c=2cp+h): each stream c-contiguous
        pbb = pb[:, :]
        pb_src = bass.AP(tensor=pbb.tensor, offset=pbb.offset,
                         ap=[pbb.ap[0], [2, 8], [16, 32], [1, 2]])
        nc.vector.tensor_copy(
            qall[:, bb].rearrange("p (v c h) -> p v c h", v=8, h=2), pb_src)

    # ---------------- chain phases (split for interleaving) ----------------
    w_ws = {}
    wrv4s = {}

    def emit_chain_write(gi):
        gs = gi * GRP
        qall = qalls[gi]
        w_w = work.tile([P, GRP, CH], BF16, tag=f"w_w_g{gi}",
                        name=f"w_w_g{gi}")
        w_ws[gi] = (w_w, qall)
        w_chain_all(qall[:, :, 0:64], qall[:, :, 320:384],
                    pw_w[:, gs:gs + GRP], 0, gs, gi, w_w)

    def emit_chain_read(gi):
        gs = gi * GRP
        w_w, qall = w_ws[gi]

        def QV(q):
            return qall[:, :, 64 * q:64 * q + 64]

        # read-head inputs via algebra (QV: 0 k_w, 1 k_r, 2 e*k_r, 3 a,
        # 4 a*e, 5 ssm, 6 sme, 7 sme2)
        dots_r = ctile("dots_r", gi)
        t_a = ctile("alg_t", gi)
        nc.vector.scalar_tensor_tensor(t_a, QV(2), -1.0, scb3(Q_AKR, gs),
                                       op0=OP.mult, op1=OP.add)
        nc.vector.tensor_mul(t_a, w_w, t_a)
        nc.vector.tensor_add(dots_r, t_a, QV(1))

        ss_r = ctile("ss_r", gi)
        a1 = ctile("alg_a1", gi)
        nc.vector.tensor_sub(a1, QV(3), QV(6))  # sma - sme
        a2 = ctile("alg_a2", gi)
        nc.vector.scalar_tensor_tensor(a2, QV(4), -2.0, scb3(Q_AA, gs),
                                       op0=OP.mult, op1=OP.add)
        nc.vector.tensor_add(a2, a2, QV(7))  # + sme2
        h1 = ctile("alg_h1", gi)
        nc.vector.tensor_mul(h1, w_w, a2)
        nc.vector.scalar_tensor_tensor(h1, a1, 2.0, h1,
                                       op0=OP.mult, op1=OP.add)
        nc.vector.tensor_mul(h1, w_w, h1)
        nc.vector.tensor_add(ss_r, h1, QV(5))  # + ssm

        w_r = work.tile([P, GRP, CH], BF16, tag=f"w_r_g{gi}",
                        name=f"w_r_g{gi}")
        w_chain_all(dots_r, ss_r, pw_r[:, gs:gs + GRP],
                    8, gs, gi, w_r)

        # wrv4[p, bb, c, 0] = w_r ; [.., 1] = w_r*w_w  (bf16 for rv moving)
        wrv4 = work.tile([P, GRP, CH, 2], BF16, tag=f"wrv_g{gi}",
                         name=f"wrv_g{gi}")
        wrv4s[gi] = wrv4
        nc.vector.tensor_copy(wrv4[:, :, :, 0], w_r)
        wrw = ctile("wrw", gi)
        nc.vector.tensor_mul(wrw, w_r, w_w)
        nc.vector.tensor_copy(wrv4[:, :, :, 1], wrw)
        # swr[b] = sum_n w_r*w_w
        swc = gtile("swc", gi, F32)
        nc.vector.reduce_sum(swc, wrw, axis=mybir.AxisListType.X)
        ps_sw = ps_misc.tile([128, 144], F32, tag="pm")
        nc.tensor.matmul(ps_sw[0:GRP, 0:1], swc, ones_sb[:, 0:1],
                         start=True, stop=True)
        swr_c = work.tile([GRP, 1], F32, tag=f"swr_c_g{gi}",
                          name=f"swr_c_g{gi}")
        nc.vector.tensor_copy(swr_c, ps_sw[0:GRP, 0:1])
        ps_swt = ps_misc.tile([128, 144], F32, tag="pm")
        nc.tensor.transpose(ps_swt[0:1, 0:GRP], swr_c,
                            ident_sb[0:GRP, 0:GRP])
        gs2 = gi * GRP
        nc.vector.tensor_copy(swr_sb[:, gs2:gs2 + GRP], ps_swt[0:1, 0:GRP])

    def emit_rv(b):
        # rv contraction: memory chunk-pair [128, 128] stationary (FWL),
        # [w_r, w_r*w_w] for both chunks as 4-wide moving; chunk halves
        # land on partition halves of a [128, 4] accumulating PSUM.
        gi, bb = b // GRP, b % GRP
        wrv4 = wrv4s[gi]
        ps_rv = ps_rvp.tile([128, 4], F32, tag="ps_rv")
        for q in range(NPAIR):
            lhs = m16s[b][:, 2 * q:2 * q + 2, :].rearrange("p c d -> p (c d)")
            rhs = wrv4[:, bb, 2 * q:2 * q + 2, :].rearrange("p c j -> p (c j)")
            nc.tensor.matmul(ps_rv, lhs, rhs,
                             start=(q == 0), stop=(q == NPAIR - 1))
        # valid: partitions 0-63 <- cols 0:2 (chunk-even), 64-127 <- 2:4
        nc.vector.tensor_copy(r12_all[0:64, b, :], ps_rv[0:64, 0:2])
        nc.vector.tensor_copy(r12_all[64:128, b, :], ps_rv[64:128, 2:4])

    # ---------------- emission schedule (software pipeline) ----------------
    emit_batch(0)
    emit_batch(1)
    emit_batch(2)
    emit_batch(3)
    emit_chain_write(0)
    emit_batch(4)
    emit_chain_read(0)
    emit_batch(5)
    emit_rv(0)
    emit_rv(1)
    emit_batch(6)
    emit_rv(2)
    emit_rv(3)
    emit_batch(7)
    emit_chain_write(1)
    emit_chain_read(1)
    for b in range(4, 8):
        emit_rv(b)

    # ---------------- read-vector assembly (all batches) ----------------
    # rvt_full[(c',d), b] = r1 - e*r2 + a*swr  on both partition halves,
    # then one transpose and a free-axis fold of the halves.
    ps_swb = ps_misc.tile([128, 144], F32, tag="pm")
    nc.tensor.matmul(ps_swb[:, 0:BL], ones_sb[0:1, :], swr_sb,
                     start=True, stop=True)
    rvt = work.tile([128, BL], F32, tag="rvt", name="rvt")
    nc.vector.tensor_mul(rvt, e2_sb, r12_all[:, :, 1])   # e * r2
    nc.vector.tensor_sub(rvt, r12_all[:, :, 0], rvt)     # r1 - e*r2
    m3 = work.tile([128, BL], F32, tag="m3", name="m3")
    nc.vector.tensor_mul(m3, a2_sb, ps_swb[:, 0:BL])     # a * sum(wr*ww)
    nc.vector.tensor_add(rvt, rvt, m3)
    ps_rvo = ps_misc.tile([128, 144], F32, tag="pm")
    nc.tensor.transpose(ps_rvo[0:BL, 0:128], rvt, ident_sb)
    rvo_sb = work.tile([BL, 128], F32, tag="rvo_sb", name="rvo_sb")
    nc.vector.tensor_copy(rvo_sb, ps_rvo[0:BL, 0:128])
    nc.vector.tensor_add(out_sb[:, C:C + D], rvo_sb[:, 0:64],
                         rvo_sb[:, 64:128])

    nc.sync.dma_start(out=out_d[:], in_=out_sb)
    ctx.close()


# ---------------------------------------------------------------------------
# host-side driver
# ---------------------------------------------------------------------------
_NC = None


def _get_module():
    global _NC
    if _NC is None:
        _NC = _build_module()
    return _NC


def _consts():
    ident = np.eye(128, dtype=np.float32)
    onest = np.ones((128, 128), np.float32)
    permu = np.zeros((128, 128), np.float32)
    permd = np.zeros((128, 128), np.float32)
    for m in range(128):
        permu[(m + 1) % 128, m] = 1.0
        permd[(m - 1) % 128, m] = 1.0
    sel = np.zeros((32, NQ * 128), np.float32)
    for q in range(NQ):
        sel[q, q * 128:(q + 1) * 128] = 1.0
    return ident, onest, permu, permd, sel


def kernel(**inputs):
    from concourse.bass_utils import run_bass_kernel_spmd

    nc = _get_module()
    f = lambda k: np.ascontiguousarray(np.asarray(inputs[k], np.float32))

    whead = np.concatenate([
        f("Wk_r"), f("Wb_r"), f("Wg_r"), f("Ws_r"), f("Wgam_r"),
        f("Wk_w"), f("Wb_w"), f("Wg_w"), f("Ws_w"), f("Wgam_w"),
        f("We_w"), f("Wa_w")], axis=1)
    bhead = np.concatenate([
        f("bk_r"), f("bb_r"), f("bg_r"), f("bs_r"), f("bgam_r"),
        f("bk_w"), f("bb_w"), f("bg_w"), f("bs_w"), f("bgam_w"),
        f("be_w"), f("ba_w")])
    ident, onest, permu, permd, sel = _consts()

    mem = f("prev_memory")
    x = f("x")
    rv = f("prev_read_vector")
    prw = f("prev_read_weights")
    pww = f("prev_write_weights")
    shared = dict(wctrl=f("W_ctrl"), bctrl=f("b_ctrl"), whead=whead,
                  bhead=bhead, ident=ident, onest=onest, permu=permu,
                  permd=permd, sel=sel)
    in_maps = []
    for c in range(NCORES):
        sl = slice(c * BL, (c + 1) * BL)
        in_maps.append(dict(
            mem=np.ascontiguousarray(mem[sl]),
            x=np.ascontiguousarray(x[sl]),
            rv=np.ascontiguousarray(rv[sl]),
            prw=np.ascontiguousarray(prw[sl]),
            pww=np.ascontiguousarray(pww[sl]),
            **shared))
    res = run_bass_kernel_spmd(nc, in_maps, list(range(NCORES)))
    return np.concatenate([res.results[c]["out"] for c in range(NCORES)],
                          axis=0).astype(np.float32)


# revision 34
# speedup vs baseline: 1.0034x; 1.0034x over previous
"""NTM cell kernel for Trainium2 (8 NeuronCores, batch-parallel).

Strategy (per core, 8 batches):
  - prev_memory slice is cast-loaded f32->bf16 into SBUF (row-major M16).
  - The on-chip transpose to T16 runs on the TENSOR engine (128x128 bf16
    transposes into PSUM, ~1 cyc/row) instead of the DMA xbar, freeing the
    DMA pool for the HBM load; PSUM->SBUF cast copies alternate between
    the vector and scalar engines, elementwise squares (T2) between
    gpsimd and vector.
  - All O(N*D) reductions run on the tensor engine:
      * content dots + sum-of-squares streams over T16 / T2 (stationary
        rides the FWL weight path at ~0.5 cyc/col)
      * read-vector contraction with the memory chunk-pair as the
        128-col stationary and [w_r, w_r*w_w] as a 4-wide moving; the
        two chunk-halves land on partition halves and are folded after
        the final output transpose.
  - new_memory is never materialized; its dot/norm/read contributions are
    expanded algebraically in terms of streams over the ORIGINAL memory.
  - Addressing chains (softmax/gate/shift/sharpen) run on DVE/ACT/GPSIMD
    in a [128 x 64] layout (n = p*64 + c), with per-group buffers so the
    two batch-groups' chains pipeline instead of serializing.
  - Only one ACT table set is used (exp/ln); sqrt/sigmoid/tanh/softplus
    are rewritten via exp/ln so no table reloads occur.
"""

import sys

sys.path.insert(0, "/opt/trn_rl_repo")

import numpy as np

import concourse.bass as bass
import concourse.tile as tile
from concourse import mybir

F32 = mybir.dt.float32
BF16 = mybir.dt.bfloat16
AF = mybir.ActivationFunctionType
OP = mybir.AluOpType

B, N, D, C, IN, S = 64, 8192, 64, 256, 128, 3
NCORES = 8
BL = B // NCORES          # batches per core
P = 128                   # partitions
CH = N // P               # 64 chunks per batch (n = p*64 + c)
NPAIR = CH // 2           # 32 transposed tiles per batch
EPS = 1e-8

# whead column map
KR0, KR1 = 0, 64
BR, GR = 64, 65
SR0, SR1 = 66, 69
GAMR = 69
KW0, KW1 = 70, 134
BW, GW = 134, 135
SW0, SW1 = 136, 139
GAMW = 139
E0, E1 = 140, 204
A0, A1 = 204, 268
NHEAD = 268

# scalar table rows (S8 cols -> SC rows -> BC blocks of 8)
Q_BET_W, Q_G_W, Q_OMG_W, Q_SW0, Q_SW1, Q_SW2, Q_GAM_W, Q_NK2_W = range(8)
Q_BET_R, Q_G_R, Q_OMG_R, Q_SR0, Q_SR1, Q_SR2, Q_GAM_R, Q_NK2_R = range(8, 16)
Q_AKR, Q_AA = 16, 17
NQ = 18

GRP = 4  # batches per pipeline group

# ---------------------------------------------------------------------------
# workaround: the deployed walrus accepts only ONE sem-wait per instruction.
# After TileContext exits, hoist extra waits onto injected single-wait nops
# (drains on the SP engine, ENGINE_NOPs elsewhere).
# ---------------------------------------------------------------------------
import concourse.tile as tile_mod


def _split_multi_waits(nc):
    for f in nc.m.functions:
        for b in f.blocks:
            insts = b.instructions
            i = 0
            while i < len(insts):
                ins = insts[i]
                si = getattr(ins, "sync_info", None)
                if si is None or len(si.on_wait) <= 1:
                    i += 1
                    continue
                waits = list(si.on_wait)
                ins.sync_info = mybir.SyncInfo(
                    on_wait=[waits[-1]], on_update=list(si.on_update)
                )
                eng = nc.engines[ins.engine]
                new_nops = []
                for w in waits[:-1]:
                    nop = eng.isa(
                        nc.isa.Opcode.NEURON_ISA_TPB_OPCODE_NOP, {}
                    ).ins
                    nop.sync_info = mybir.SyncInfo(on_wait=[w], on_update=[])
                    new_nops.append(nop)
                for nop in new_nops:
                    for bb2 in f.blocks:
                        try:
                            bb2.instructions.remove(nop)
                            break
                        except ValueError:
                            pass
                for k, nop in enumerate(new_nops):
                    insts.insert(i + k, nop)
                i += len(new_nops) + 1


_orig_exit = tile_mod.TileContext.__exit__


def _patched_exit(self, *a, **k):
    import os
    r = _orig_exit(self, *a, **k)
    if not os.environ.get("NTM_NO_WAITFIX"):
        _split_multi_waits(self.nc)
    return r


if not getattr(tile_mod.TileContext, "_waitfix_patched", False):
    tile_mod.TileContext.__exit__ = _patched_exit
    tile_mod.TileContext._waitfix_patched = True


# ---------------------------------------------------------------------------
# kernel body
# ---------------------------------------------------------------------------

def _build_module():
    nc = bass.Bass()

    mem = nc.dram_tensor("mem", [BL, N, D], F32, kind="ExternalInput")
    x_in = nc.dram_tensor("x", [BL, IN], F32, kind="ExternalInput")
    rv_in = nc.dram_tensor("rv", [BL, D], F32, kind="ExternalInput")
    prw_in = nc.dram_tensor("prw", [BL, N], F32, kind="ExternalInput")
    pww_in = nc.dram_tensor("pww", [BL, N], F32, kind="ExternalInput")
    wctrl = nc.dram_tensor("wctrl", [IN + D, C], F32, kind="ExternalInput")
    bctrl = nc.dram_tensor("bctrl", [C], F32, kind="ExternalInput")
    whead = nc.dram_tensor("whead", [C, NHEAD], F32, kind="ExternalInput")
    bhead = nc.dram_tensor("bhead", [NHEAD], F32, kind="ExternalInput")
    ident = nc.dram_tensor("ident", [128, 128], F32, kind="ExternalInput")
    onest = nc.dram_tensor("onest", [128, 128], F32, kind="ExternalInput")
    permu = nc.dram_tensor("permu", [128, 128], F32, kind="ExternalInput")
    permd = nc.dram_tensor("permd", [128, 128], F32, kind="ExternalInput")
    seldr = nc.dram_tensor("sel", [32, NQ * 128], F32, kind="ExternalInput")
    out_d = nc.dram_tensor("out", [BL, C + D], F32, kind="ExternalOutput")

    with tile.TileContext(nc) as tc:
        _emit(nc, tc, mem, x_in, rv_in, prw_in, pww_in, wctrl, bctrl, whead,
              bhead, ident, onest, permu, permd, seldr, out_d)
    return nc


def _emit(nc, tc, mem, x_in, rv_in, prw_in, pww_in, wctrl, bctrl, whead,
          bhead, ident, onest, permu, permd, seldr, out_d):
    from contextlib import ExitStack

    ctx = ExitStack()
    ctx.enter_context(nc.allow_low_precision(
        reason="bf16 chain intermediates; rel-err budget 2e-2"))
    big = ctx.enter_context(tc.tile_pool(name="big", bufs=1))
    cons = ctx.enter_context(tc.tile_pool(name="cons", bufs=1))
    work = ctx.enter_context(tc.tile_pool(name="work", bufs=1))
    t16p = ctx.enter_context(tc.tile_pool(name="t16p", bufs=3))
    t2p = ctx.enter_context(tc.tile_pool(name="t2p", bufs=2))
    qallp = ctx.enter_context(tc.tile_pool(name="qallp", bufs=2))
    ps_tr = ctx.enter_context(tc.tile_pool(name="ps_tr", bufs=2, space="PSUM"))
    ps_stream = ctx.enter_context(tc.tile_pool(name="ps_stream", bufs=2, space="PSUM"))
    ps_misc = ctx.enter_context(tc.tile_pool(name="ps_misc", bufs=3, space="PSUM"))
    ps_rvp = ctx.enter_context(tc.tile_pool(name="ps_rvp", bufs=1, space="PSUM"))

    # ---------------- big memory load: issue FIRST ----------------
    # batch 0 on sync so its descriptors hit the rings first; the rest
    # sequentially behind it on gpsimd (per-ring FIFO keeps completion
    # roughly batch-ordered).
    # bf16 const cast-loads go FIRST on the gpsimd SW queue (tiny), then
    # the 8 per-batch memory cast-loads (f32->bf16 in the DGE) behind them.
    m16s = [big.tile([P, CH, D], BF16, tag=f"m16_{b}", name=f"m16_{b}")
            for b in range(BL)]

    identb_sb = cons.tile([128, 128], BF16, tag="identb")
    nc.gpsimd.dma_start(out=identb_sb, in_=ident[:])
    wh0 = cons.tile([128, NHEAD], BF16, tag="wh0")
    nc.gpsimd.dma_start(out=wh0, in_=whead[0:128, :])
    wh1 = cons.tile([128, NHEAD], BF16, tag="wh1")
    nc.gpsimd.dma_start(out=wh1, in_=whead[128:256, :])
    bh_sb = cons.tile([1, NHEAD], BF16, tag="bh")
    nc.gpsimd.dma_start(out=bh_sb, in_=bhead.rearrange("(o n) -> o n", o=1))
    selb_sb = cons.tile([32, NQ * 128], BF16, tag="selb")
    nc.gpsimd.dma_start(out=selb_sb, in_=seldr[:])
    onesb_sb = cons.tile([1, 128], BF16, tag="onesb")
    nc.gpsimd.dma_start(out=onesb_sb, in_=onest[0:1, :])
    permub_sb = cons.tile([128, 128], BF16, tag="permub")
    nc.gpsimd.dma_start(out=permub_sb, in_=permu[:])
    permdb_sb = cons.tile([128, 128], BF16, tag="permdb")
    nc.gpsimd.dma_start(out=permdb_sb, in_=permd[:])
    for b in range(BL):
        nc.gpsimd.dma_start(
            out=m16s[b], in_=mem[b].rearrange("(p c) d -> p c d", p=128)
        )

    wc0 = cons.tile([128, C], F32, tag="wc0")
    nc.sync.dma_start(out=wc0, in_=wctrl[0:128, :])
    wc1 = cons.tile([64, C], F32, tag="wc1")
    nc.sync.dma_start(out=wc1, in_=wctrl[128:192, :])
    bc_sb = cons.tile([128, 2], F32, tag="bc")
    nc.sync.dma_start(out=bc_sb, in_=bctrl.rearrange("(j p) -> p j", p=128))
    xt_in = cons.tile([BL, IN], F32, tag="xt_in")
    nc.sync.dma_start(out=xt_in, in_=x_in[:])
    rv_sb = cons.tile([BL, D], F32, tag="rv_sb")
    nc.sync.dma_start(out=rv_sb, in_=rv_in[:])
    ident_sb = cons.tile([128, 128], F32, tag="ident")
    nc.sync.dma_start(out=ident_sb, in_=ident[:])

    ones_sb = cons.tile([128, 128], F32, tag="ones")
    nc.scalar.dma_start(out=ones_sb, in_=onest[:])
    permu_sb = cons.tile([128, 128], F32, tag="permu")
    nc.scalar.dma_start(out=permu_sb, in_=permu[:])
    permd_sb = cons.tile([128, 128], F32, tag="permd")
    nc.scalar.dma_start(out=permd_sb, in_=permd[:])
    pw_w = cons.tile([128, BL, CH], F32, tag="pw_w")
    nc.scalar.dma_start(out=pw_w, in_=pww_in.rearrange("b (p c) -> p b c", p=128))
    pw_r = cons.tile([128, BL, CH], F32, tag="pw_r")
    nc.scalar.dma_start(out=pw_r, in_=prw_in.rearrange("b (p c) -> p b c", p=128))


    # ---------------- controller: hT = relu(W_ctrl^T @ ctrl_in^T + b) -------
    ps_xt = ps_misc.tile([128, 144], F32, tag="pm")
    nc.tensor.transpose(ps_xt[:, 0:BL], xt_in, ident_sb[0:BL, 0:BL])
    xT = work.tile([128, BL], F32, tag="xT")
    nc.vector.tensor_copy(xT, ps_xt[:, 0:BL])
    ps_rt = ps_misc.tile([128, 144], F32, tag="pm")
    nc.tensor.transpose(ps_rt[0:D, 0:BL], rv_sb, ident_sb[0:BL, 0:BL])
    rvT = work.tile([64, BL], F32, tag="rvT")
    nc.vector.tensor_copy(rvT, ps_rt[0:D, 0:BL])

    hT_sb = []
    for j in range(2):
        ps_h = ps_misc.tile([128, 144], F32, tag="pm")
        nc.tensor.matmul(ps_h[:, 0:BL], wc0[:, j * 128:(j + 1) * 128], xT,
                         start=True, stop=False)
        nc.tensor.matmul(ps_h[:, 0:BL], wc1[:, j * 128:(j + 1) * 128], rvT,
                         start=False, stop=True)
        h_j = work.tile([128, BL], F32, tag=f"hT{j}")
        nc.scalar.activation(h_j, ps_h[:, 0:BL], AF.Relu,
                             bias=bc_sb[:, j:j + 1], scale=1.0)
        hT_sb.append(h_j)

    # ---------------- head params P = h @ Whead + bhead (bf16) ----------
    hT_b = []
    for j in range(2):
        hb = work.tile([128, BL], BF16, tag=f"hTb{j}", name=f"hTb{j}")
        nc.vector.tensor_copy(hb, hT_sb[j])
        hT_b.append(hb)
    ps_p = ps_misc.tile([BL, 512], F32, tag="pm")
    nc.tensor.matmul(ps_p[:, 0:NHEAD], hT_b[0], wh0, start=True, stop=False)
    nc.tensor.matmul(ps_p[:, 0:NHEAD], hT_b[1], wh1, start=False, stop=False)
    nc.tensor.matmul(ps_p[:, 0:NHEAD], onesb_sb[0:1, 0:BL], bh_sb,
                     start=False, stop=True)
    p_sb = work.tile([BL, NHEAD], F32, tag="p_sb")
    nc.vector.tensor_copy(p_sb, ps_p[:, 0:NHEAD])

    # ---------------- VA: per-batch d-vectors [BL, 8*64] ----------------
    # vec order: 0 k_w, 1 k_r, 2 e*k_r, 3 a, 4 a*e, 5 ones, 6 e, 7 e^2
    va = work.tile([BL, 512], F32, tag="va")
    nc.vector.tensor_copy(va[:, 0:64], p_sb[:, KW0:KW1])
    nc.vector.tensor_copy(va[:, 64:128], p_sb[:, KR0:KR1])

    def _sigmoid(dst, src):  # dst = 1/(1+exp(-src))
        nc.scalar.activation(dst, src, AF.Exp, scale=-1.0)
        nc.vector.tensor_scalar_add(dst, dst, 1.0)
        nc.vector.reciprocal(dst, dst)

    # e = sigmoid(P_e) -> va[:, 384:448]
    _sigmoid(va[:, 384:448], p_sb[:, E0:E1])
    # a = tanh(P_a) = 1 - 2/(exp(2x)+1) -> va[:, 192:256]
    nc.scalar.activation(va[:, 192:256], p_sb[:, A0:A1], AF.Exp, scale=2.0)
    nc.vector.tensor_scalar_add(va[:, 192:256], va[:, 192:256], 1.0)
    nc.vector.reciprocal(va[:, 192:256], va[:, 192:256])
    nc.vector.tensor_scalar(va[:, 192:256], va[:, 192:256], -2.0, 1.0,
                            op0=OP.mult, op1=OP.add)
    # beta' = softplus(P_beta) * rsqrt(||k||^2): folded into the stream
    # vectors so the chain's beta-mul and nk2-normalization disappear.
    tmp64p = work.tile([BL, 64], F32, tag="tmp64p")
    bp = work.tile([BL, 4], F32, tag="bp")
    nc.vector.tensor_mul(tmp64p, va[:, 0:64], va[:, 0:64])
    nc.vector.reduce_sum(bp[:, 0:1], tmp64p, axis=mybir.AxisListType.X)
    nc.vector.tensor_mul(tmp64p, va[:, 64:128], va[:, 64:128])
    nc.vector.reduce_sum(bp[:, 1:2], tmp64p, axis=mybir.AxisListType.X)
    nc.scalar.activation(bp[:, 0:2], bp[:, 0:2], AF.Ln)
    nc.scalar.activation(bp[:, 0:2], bp[:, 0:2], AF.Exp, scale=-0.5)
    nc.scalar.activation(bp[:, 2:3], p_sb[:, BW:BW + 1], AF.Exp)
    nc.scalar.activation(bp[:, 3:4], p_sb[:, BR:BR + 1], AF.Exp)
    nc.vector.tensor_scalar_add(bp[:, 2:4], bp[:, 2:4], 1.0)
    nc.scalar.activation(bp[:, 2:4], bp[:, 2:4], AF.Ln)
    nc.vector.tensor_mul(bp[:, 2:3], bp[:, 2:3], bp[:, 0:1])
    nc.vector.tensor_mul(bp[:, 3:4], bp[:, 3:4], bp[:, 1:2])
    nc.vector.tensor_scalar(va[:, 0:64], va[:, 0:64], bp[:, 2:3], None,
                            op0=OP.mult)
    nc.vector.tensor_scalar(va[:, 64:128], va[:, 64:128], bp[:, 3:4], None,
                            op0=OP.mult)
    # e*k_r (scaled), a*e, ones, e^2
    nc.vector.tensor_mul(va[:, 128:192], va[:, 384:448], va[:, 64:128])
    nc.vector.tensor_mul(va[:, 256:320], va[:, 192:256], va[:, 384:448])
    nc.vector.memset(va[:, 320:384], 1.0)
    nc.vector.tensor_mul(va[:, 448:512], va[:, 384:448], va[:, 384:448])

    # ---------------- VTD: transposed vectors with zero-halves --------------
    # VTD[p, half, vec, b]; half 0: rows 0-63 hold vec, rows 64-127 zero.
    vtd = work.tile([128, 2, 8, BL], BF16, tag="vtd")
    nc.vector.memset(vtd, 0.0)
    vapad = work.tile([BL, 8, 128], F32, tag="vapad")
    nc.vector.memset(vapad, 0.0)
    for v in range(8):
        nc.vector.tensor_copy(vapad[:, v, 64:128], va[:, v * 64:(v + 1) * 64])
    ps_top = ps_misc.tile([128, 144], F32, tag="pm")
    ps_bot = ps_misc.tile([128, 144], F32, tag="pm")
    for v in range(8):
        nc.tensor.transpose(ps_top[0:64, v * BL:(v + 1) * BL],
                            va[:, v * 64:(v + 1) * 64],
                            ident_sb[0:BL, 0:BL])
        nc.tensor.transpose(ps_bot[:, v * BL:(v + 1) * BL],
                            vapad[:, v, :], ident_sb[0:BL, 0:BL])
    nc.vector.tensor_copy(
        vtd[0:64].rearrange("p h v b -> p (h v b)")[:, 0:64],
        ps_top[0:64, 0:64])
    nc.vector.tensor_copy(
        vtd[64:128].rearrange("p h v b -> p (h v b)")[:, 64:128],
        ps_bot[64:128, 0:64])
    # e/a duplicated across both partition halves for the rv assembly:
    # ea_dup[b, 0:64] = e, [64:128] = e  (same for a) -> transpose -> [128, BL]
    ea_dup = work.tile([BL, 2, 128], F32, tag="ea_dup")
    nc.vector.tensor_copy(ea_dup[:, 0, 0:64], va[:, 384:448])
    nc.vector.tensor_copy(ea_dup[:, 0, 64:128], va[:, 384:448])
    nc.vector.tensor_copy(ea_dup[:, 1, 0:64], va[:, 192:256])
    nc.vector.tensor_copy(ea_dup[:, 1, 64:128], va[:, 192:256])
    ps_ea = ps_misc.tile([128, 144], F32, tag="pm")
    nc.tensor.transpose(ps_ea[:, 0:BL], ea_dup[:, 0, :], ident_sb[0:BL, 0:BL])
    nc.tensor.transpose(ps_ea[:, BL:2 * BL], ea_dup[:, 1, :],
                        ident_sb[0:BL, 0:BL])
    e2_sb = work.tile([128, BL], F32, tag="e2_sb")
    nc.vector.tensor_copy(e2_sb, ps_ea[:, 0:BL])
    a2_sb = work.tile([128, BL], F32, tag="a2_sb")
    nc.vector.tensor_copy(a2_sb, ps_ea[:, BL:2 * BL])

    # ---------------- per-batch scalars S8 [BL, 32] ----------------
    s8 = work.tile([BL, 32], F32, tag="s8")
    nc.vector.memset(s8, 0.0)
    tmp64 = work.tile([BL, 64], F32, tag="tmp64")

    def _softplus(dst, src):  # ln(1 + exp(src))
        nc.scalar.activation(dst, src, AF.Exp)
        nc.vector.tensor_scalar_add(dst, dst, 1.0)
        nc.scalar.activation(dst, dst, AF.Ln)

    def _softmax3(dst, src):
        ex3 = work.tile([BL, 3], F32, tag="ex3")
        nc.scalar.activation(ex3, src, AF.Exp)
        sm = work.tile([BL, 1], F32, tag="sm3")
        nc.vector.reduce_sum(sm, ex3, axis=mybir.AxisListType.X)
        nc.vector.reciprocal(sm, sm)
        nc.vector.tensor_scalar(dst, ex3, sm, None, op0=OP.mult)

    _softplus(s8[:, Q_BET_W:Q_BET_W + 1], p_sb[:, BW:BW + 1])
    _sigmoid(s8[:, Q_G_W:Q_G_W + 1], p_sb[:, GW:GW + 1])
    nc.vector.tensor_scalar(s8[:, Q_OMG_W:Q_OMG_W + 1],
                            s8[:, Q_G_W:Q_G_W + 1], -1.0, 1.0,
                            op0=OP.mult, op1=OP.add)
    _softmax3(s8[:, Q_SW0:Q_SW0 + 3], p_sb[:, SW0:SW1])
    _softplus(s8[:, Q_GAM_W:Q_GAM_W + 1], p_sb[:, GAMW:GAMW + 1])
    nc.vector.tensor_scalar_add(s8[:, Q_GAM_W:Q_GAM_W + 1],
                                s8[:, Q_GAM_W:Q_GAM_W + 1], 1.0)
    nc.vector.tensor_mul(tmp64, va[:, 0:64], va[:, 0:64])
    nc.vector.reduce_sum(s8[:, Q_NK2_W:Q_NK2_W + 1], tmp64,
                         axis=mybir.AxisListType.X)

    _softplus(s8[:, Q_BET_R:Q_BET_R + 1], p_sb[:, BR:BR + 1])
    _sigmoid(s8[:, Q_G_R:Q_G_R + 1], p_sb[:, GR:GR + 1])
    nc.vector.tensor_scalar(s8[:, Q_OMG_R:Q_OMG_R + 1],
                            s8[:, Q_G_R:Q_G_R + 1], -1.0, 1.0,
                            op0=OP.mult, op1=OP.add)
    _softmax3(s8[:, Q_SR0:Q_SR0 + 3], p_sb[:, SR0:SR1])
    _softplus(s8[:, Q_GAM_R:Q_GAM_R + 1], p_sb[:, GAMR:GAMR + 1])
    nc.vector.tensor_scalar_add(s8[:, Q_GAM_R:Q_GAM_R + 1],
                                s8[:, Q_GAM_R:Q_GAM_R + 1], 1.0)
    nc.vector.tensor_mul(tmp64, va[:, 64:128], va[:, 64:128])
    nc.vector.reduce_sum(s8[:, Q_NK2_R:Q_NK2_R + 1], tmp64,
                         axis=mybir.AxisListType.X)

    nc.vector.tensor_mul(tmp64, va[:, 192:256], va[:, 64:128])
    nc.vector.reduce_sum(s8[:, Q_AKR:Q_AKR + 1], tmp64,
                         axis=mybir.AxisListType.X)
    nc.vector.tensor_mul(tmp64, va[:, 192:256], va[:, 192:256])
    nc.vector.reduce_sum(s8[:, Q_AA:Q_AA + 1], tmp64,
                         axis=mybir.AxisListType.X)

    # transpose S8 -> SC [32, BL] and broadcast -> BC [128, NQ*8]
    ps_sc = ps_misc.tile([128, 144], F32, tag="pm")
    nc.tensor.transpose(ps_sc[0:32, 0:BL], s8, ident_sb[0:BL, 0:BL])
    sc_sb = work.tile([32, BL], BF16, tag="sc_sb")
    nc.vector.tensor_copy(sc_sb, ps_sc[0:32, 0:BL])
    ps_bc = ps_misc.tile([128, 144], F32, tag="pm")
    for q in range(NQ):
        nc.tensor.matmul(ps_bc[:, q * BL:(q + 1) * BL],
                         selb_sb[:, q * 128:(q + 1) * 128], sc_sb,
                         start=True, stop=True)
    bc_all = work.tile([128, NQ * BL], BF16, tag="bc_all")
    nc.vector.tensor_copy(bc_all, ps_bc[:, 0:NQ * BL])
    bc_f32 = work.tile([128, NQ * BL], F32, tag="bc_f32")
    nc.vector.tensor_copy(bc_f32, ps_bc[:, 0:NQ * BL])

    def BCF(q, b):
        return bc_f32[:, q * BL + b:q * BL + b + 1]

    def BC(q, b):
        return bc_all[:, q * BL + b:q * BL + b + 1]

    # ---------------- output staging ----------------
    out_sb = work.tile([BL, C + D], F32, tag="out_sb")
    ps_ho = ps_misc.tile([128, 144], F32, tag="pm")
    nc.tensor.transpose(ps_ho[0:BL, 0:128], hT_sb[0], ident_sb)
    nc.vector.tensor_copy(out_sb[:, 0:128], ps_ho[0:BL, 0:128])
    ps_ho2 = ps_misc.tile([128, 144], F32, tag="pm")
    nc.tensor.transpose(ps_ho2[0:BL, 0:128], hT_sb[1], ident_sb)
    nc.vector.tensor_copy(out_sb[:, 128:256], ps_ho2[0:BL, 0:128])

    swr_sb = work.tile([1, BL], F32, tag="swr_sb")
    r12_all = work.tile([128, BL, 2], F32, tag="r12_all")

    # ---------------- helpers for grouped heavy phase ----------------
    def scb3(q, gs):
        base = bc_all[:, q * BL + gs:q * BL + gs + GRP]
        return bass.AP(tensor=base.tensor, offset=base.offset,
                       ap=[base.ap[0], base.ap[1], [0, CH]])

    def scb3n(q, gs, n):
        base = bc_all[:, q * BL + gs:q * BL + gs + GRP]
        return bass.AP(tensor=base.tensor, offset=base.offset,
                       ap=[base.ap[0], base.ap[1], [0, n]])

    def bc3(t8):
        base = t8[:, :]
        return bass.AP(tensor=base.tensor, offset=base.offset,
                       ap=[base.ap[0], base.ap[1], [0, CH]])

    def ctile(tag, gi):
        tg = f"{tag}_g{gi}"
        return work.tile([P, GRP, CH], BF16, tag=tg, name=tg)

    def gtile(tag, gi, dt=F32):
        tg = f"{tag}_g{gi}"
        return work.tile([128, GRP], dt, tag=tg, name=tg)

    def psum_colsum_bcast(cs8, gi, eps=None, tag="tot"):
        # one matmul with a full ones stationary both sums over partitions
        # and broadcasts the per-batch total to every output partition
        ps_t = ps_misc.tile([128, 144], F32, tag="pm")
        nc.tensor.matmul(ps_t[:, 0:GRP], ones_sb, cs8, start=True, stop=True)
        rt = gtile(tag + "_rt", gi)
        if eps is not None:
            nc.vector.tensor_scalar_add(rt, ps_t[:, 0:GRP], eps)
            nc.vector.reciprocal(rt, rt)
        else:
            nc.vector.reciprocal(rt, ps_t[:, 0:GRP])
        return rt

    def w_chain_all(dk_v, ssm_v, pw_all, qo, gs, gi, dst):
        bet, g_, omg, s0, s1, s2, gam, nk2 = (qo + i for i in range(8))
        v = ctile("wc_v", gi)
        nc.scalar.activation(v, ssm_v, AF.Ln)
        inv = ctile("wc_inv", gi)
        nc.scalar.activation(inv, v, AF.Exp, scale=-0.5)
        bsim = ctile("wc_bsim", gi)
        nc.vector.tensor_mul(bsim, dk_v, inv)
        # exp + per-partition row-sum fused on ACT, one per batch
        ex = ctile("wc_ex", gi)
        cs = gtile("wc_cs", gi, F32)
        for j in range(GRP):
            nc.scalar.activation(ex[:, j], bsim[:, j], AF.Exp,
                                 accum_out=cs[:, j:j + 1])
        # sharpening is scale-invariant, so fold the content-softmax
        # denominator T into the interpolation instead of normalizing:
        # ws' = g*ex + T*(1-g)*pw  (T broadcast by the colsum matmul)
        ps_T = ps_misc.tile([128, 144], F32, tag="pm")
        nc.tensor.matmul(ps_T[:, 0:GRP], ones_sb, cs, start=True, stop=True)
        omgT = gtile("wc_omgT", gi)
        nc.vector.tensor_mul(omgT, ps_T[:, 0:GRP],
                             bc_all[:, omg * BL + gs:omg * BL + gs + GRP])
        t9 = ctile("wc_t9", gi)
        nc.vector.tensor_mul(t9, pw_all, bc3(omgT))
        wg = ctile("wc_wg", gi)
        for j in range(GRP):
            nc.scalar.activation(wg[:, j], ex[:, j], AF.Copy,
                                 scale=BCF(g_, gs + j))
        nc.vector.tensor_add(wg, wg, t9)
        # circular shift: body via shifted APs, boundary cols via perm matmuls
        ps_sh = ps_misc.tile([128, 144], F32, tag="pm")
        nc.tensor.matmul(ps_sh[:, 0:GRP], permub_sb, wg[:, :, 0],
                         start=True, stop=True)
        nc.tensor.matmul(ps_sh[:, GRP:2 * GRP], permdb_sb, wg[:, :, CH - 1],
                         start=True, stop=True)
        ws = ctile("wc_ws", gi)
        for j in range(GRP):
            nc.scalar.activation(ws[:, j], wg[:, j], AF.Copy,
                                 scale=BCF(s1, gs + j))
        tA = ctile("wc_tA", gi)
        nc.vector.tensor_mul(tA[:, :, 0:CH - 1], wg[:, :, 1:CH],
                             scb3n(s0, gs, CH - 1))
        nc.vector.tensor_add(ws[:, :, 0:CH - 1], ws[:, :, 0:CH - 1],
                             tA[:, :, 0:CH - 1])
        nc.vector.tensor_mul(tA[:, :, 1:CH], wg[:, :, 0:CH - 1],
                             scb3n(s2, gs, CH - 1))
        nc.vector.tensor_add(ws[:, :, 1:CH], ws[:, :, 1:CH],
                             tA[:, :, 1:CH])
        bnd = work.tile([128, 2 * GRP], F32, tag=f"wc_bnd_g{gi}",
                        name=f"wc_bnd_g{gi}")
        nc.vector.tensor_mul(bnd[:, 0:GRP], ps_sh[:, 0:GRP],
                             bc_all[:, s0 * BL + gs:s0 * BL + gs + GRP])
        nc.vector.tensor_mul(bnd[:, GRP:2 * GRP], ps_sh[:, GRP:2 * GRP],
                             bc_all[:, s2 * BL + gs:s2 * BL + gs + GRP])
        nc.vector.tensor_add(ws[:, :, CH - 1], ws[:, :, CH - 1],
                             bnd[:, 0:GRP])
        nc.vector.tensor_add(ws[:, :, 0], ws[:, :, 0], bnd[:, GRP:2 * GRP])
        # sharpening: wp = exp(gam * ln(ws)) with fused row-sums
        lg = ctile("wc_lg", gi)
        nc.scalar.activation(lg, ws, AF.Ln)
        wp = ctile("wc_wp", gi)
        cs2 = gtile("wc_cs2", gi, F32)
        for j in range(GRP):
            nc.scalar.activation(wp[:, j], lg[:, j], AF.Exp,
                                 scale=BCF(gam, gs + j),
                                 accum_out=cs2[:, j:j + 1])
        rt2 = psum_colsum_bcast(cs2, gi, eps=EPS, tag="wc_t2")
        for j in range(GRP):
            nc.scalar.activation(dst[:, j], wp[:, j], AF.Copy,
                                 scale=rt2[:, j:j + 1])

    # ---------------- per-batch heavy stream ----------------
    qalls = {}

    def emit_batch(b):
        gi, bb = b // GRP, b % GRP
        if bb == 0:
            qalls[gi] = qallp.tile([P, GRP, 512], BF16, tag="qall",
                                   name="qall")
        qall = qalls[gi]
        t16b = t16p.tile([P, NPAIR, 128], BF16, tag="t16b", name="t16b")
        t2b = t2p.tile([P, NPAIR, 128], BF16, tag="t2b", name="t2b")
        m16f = m16s[b].rearrange("p c d -> p (c d)")
        if b == BL - 1:
            # the DMA pool is idle once the load drains: xbar-transpose the
            # last batch to take its transposes+copies off the PE/ACT path
            nc.sync.dma_start_transpose(t16b[:, 0:16], m16f[:, 0:2048])
            nc.scalar.dma_start_transpose(t16b[:, 16:32], m16f[:, 2048:4096])
            big0 = t16b[:, 0:16].rearrange("p a q -> p (a q)")
            big0d = t2b[:, 0:16].rearrange("p a q -> p (a q)")
            nc.gpsimd.tensor_mul(big0d, big0, big0)
            big1 = t16b[:, 16:32].rearrange("p a q -> p (a q)")
            big1d = t2b[:, 16:32].rearrange("p a q -> p (a q)")
            nc.vector.tensor_mul(big1d, big1, big1)
            _emit_streams(b, bb, qall, t16b, t2b)
            return
        # transposes in quads sharing one PSUM bank; copies alternate
        # vector/scalar, squares alternate gpsimd/vector
        for q in range(4):
            ps_t = ps_tr.tile([128, 1024], BF16, tag="ps_t")
            for k in range(8):
                cp = q * 8 + k
                nc.tensor.transpose(ps_t[:, k * 128:(k + 1) * 128],
                                    m16f[:, cp * 128:(cp + 1) * 128],
                                    identb_sb)
            t16v = t16b[:, q * 8:(q + 1) * 8].rearrange("p a q -> p (a q)")
            t2v = t2b[:, q * 8:(q + 1) * 8].rearrange("p a q -> p (a q)")
            if q == 2:
                nc.vector.tensor_copy(t16v, ps_t)
            else:
                nc.scalar.activation(t16v, ps_t, AF.Copy)
            if q == 1:
                big2 = t16b[:, 0:16].rearrange("p a q -> p (a q)")
                big2d = t2b[:, 0:16].rearrange("p a q -> p (a q)")
                nc.gpsimd.tensor_mul(big2d, big2, big2)
            elif q >= 2:
                nc.vector.tensor_mul(t2v, t16v, t16v)
        _emit_streams(b, bb, qall, t16b, t2b)

    def _emit_streams(b, bb, qall, t16b, t2b):
        pb = ps_stream.tile([128, 512], F32, tag="pb")
        rhs_m = vtd[:, :, 0:5, b].rearrange("p h v -> p v h")
        rhs_s = vtd[:, :, 5:8, b].rearrange("p h v -> p v h")
        for cp in range(NPAIR):
            nc.tensor.matmul(pb[:, cp * 16:cp * 16 + 10],
                             t16b[:, cp], rhs_m, start=True, stop=True)
        for cp in range(NPAIR):
            nc.tensor.matmul(pb[:, cp * 16 + 10:cp * 16 + 16],
                             t2b[:, cp], rhs_s, start=True, stop=True)
        # de-interleave (cp, 2v+h) -> (v, c=2cp+h): each stream c-contiguous
        pbb = pb[:, :]
        pb_src = bass.AP(tensor=pbb.tensor, offset=pbb.offset,
                         ap=[pbb.ap[0], [2, 8], [16, 32], [1, 2]])
        nc.vector.tensor_copy(
            qall[:, bb].rearrange("p (v c h) -> p v c h", v=8, h=2), pb_src)

    # ---------------- chain phases (split for interleaving) ----------------
    w_ws = {}
    wrv4s = {}

    def emit_chain_write(gi):
        gs = gi * GRP
        qall = qalls[gi]
        w_w = work.tile([P, GRP, CH], BF16, tag=f"w_w_g{gi}",
                        name=f"w_w_g{gi}")
        w_ws[gi] = (w_w, qall)
        w_chain_all(qall[:, :, 0:64], qall[:, :, 320:384],
                    pw_w[:, gs:gs + GRP], 0, gs, gi, w_w)

    def emit_chain_read(gi):
        gs = gi * GRP
        w_w, qall = w_ws[gi]

        def QV(q):
            return qall[:, :, 64 * q:64 * q + 64]

        # read-head inputs via algebra (QV: 0 k_w, 1 k_r, 2 e*k_r, 3 a,
        # 4 a*e, 5 ssm, 6 sme, 7 sme2)
        dots_r = ctile("dots_r", gi)
        t_a = ctile("alg_t", gi)
        nc.vector.scalar_tensor_tensor(t_a, QV(2), -1.0, scb3(Q_AKR, gs),
                                       op0=OP.mult, op1=OP.add)
        nc.vector.tensor_mul(t_a, w_w, t_a)
        nc.vector.tensor_add(dots_r, t_a, QV(1))

        ss_r = ctile("ss_r", gi)
        a1 = ctile("alg_a1", gi)
        nc.vector.tensor_sub(a1, QV(3), QV(6))  # sma - sme
        a2 = ctile("alg_a2", gi)
        nc.vector.scalar_tensor_tensor(a2, QV(4), -2.0, scb3(Q_AA, gs),
                                       op0=OP.mult, op1=OP.add)
        nc.vector.tensor_add(a2, a2, QV(7))  # + sme2
        h1 = ctile("alg_h1", gi)
        nc.vector.tensor_mul(h1, w_w, a2)
        nc.vector.scalar_tensor_tensor(h1, a1, 2.0, h1,
                                       op0=OP.mult, op1=OP.add)
        nc.vector.tensor_mul(h1, w_w, h1)
        nc.vector.tensor_add(ss_r, h1, QV(5))  # + ssm

        w_r = work.tile([P, GRP, CH], BF16, tag=f"w_r_g{gi}",
                        name=f"w_r_g{gi}")
        w_chain_all(dots_r, ss_r, pw_r[:, gs:gs + GRP],
                    8, gs, gi, w_r)

        # wrv4[p, bb, c, 0] = w_r ; [.., 1] = w_r*w_w  (bf16 for rv moving)
        wrv4 = work.tile([P, GRP, CH, 2], BF16, tag=f"wrv_g{gi}",
                         name=f"wrv_g{gi}")
        wrv4s[gi] = wrv4
        nc.vector.tensor_copy(wrv4[:, :, :, 0], w_r)
        wrw = ctile("wrw", gi)
        nc.vector.tensor_mul(wrw, w_r, w_w)
        nc.vector.tensor_copy(wrv4[:, :, :, 1], wrw)
        # swr[b] = sum_n w_r*w_w
        swc = gtile("swc", gi, F32)
        nc.vector.reduce_sum(swc, wrw, axis=mybir.AxisListType.X)
        ps_sw = ps_misc.tile([128, 144], F32, tag="pm")
        nc.tensor.matmul(ps_sw[0:GRP, 0:1], swc, ones_sb[:, 0:1],
                         start=True, stop=True)
        swr_c = work.tile([GRP, 1], F32, tag=f"swr_c_g{gi}",
                          name=f"swr_c_g{gi}")
        nc.vector.tensor_copy(swr_c, ps_sw[0:GRP, 0:1])
        ps_swt = ps_misc.tile([128, 144], F32, tag="pm")
        nc.tensor.transpose(ps_swt[0:1, 0:GRP], swr_c,
                            ident_sb[0:GRP, 0:GRP])
        gs2 = gi * GRP
        nc.vector.tensor_copy(swr_sb[:, gs2:gs2 + GRP], ps_swt[0:1, 0:GRP])

    def emit_rv(b):
        # rv contraction: memory chunk-pair [128, 128] stationary (FWL),
        # [w_r, w_r*w_w] for both chunks as 4-wide moving; chunk halves
        # land on partition halves of a [128, 4] accumulating PSUM.
        gi, bb = b // GRP, b % GRP
        wrv4 = wrv4s[gi]
        ps_rv = ps_rvp.tile([128, 4], F32, tag="ps_rv")
        for q in range(NPAIR):
            lhs = m16s[b][:, 2 * q:2 * q + 2, :].rearrange("p c d -> p (c d)")
            rhs = wrv4[:, bb, 2 * q:2 * q + 2, :].rearrange("p c j -> p (c j)")
            nc.tensor.matmul(ps_rv, lhs, rhs,
                             start=(q == 0), stop=(q == NPAIR - 1))
        # valid: partitions 0-63 <- cols 0:2 (chunk-even), 64-127 <- 2:4
        nc.vector.tensor_copy(r12_all[0:64, b, :], ps_rv[0:64, 0:2])
        nc.vector.tensor_copy(r12_all[64:128, b, :], ps_rv[64:128, 2:4])

    # ---------------- emission schedule (software pipeline) ----------------
    emit_batch(0)
    emit_batch(1)
    emit_batch(2)
    emit_batch(3)
    emit_chain_write(0)
    emit_batch(4)
    emit_chain_read(0)
    emit_batch(5)
    emit_rv(0)
    emit_rv(1)
    emit_batch(6)
    emit_rv(2)
    emit_rv(3)
    emit_batch(7)
    emit_chain_write(1)
    emit_chain_read(1)
    for b in range(4, 8):
        emit_rv(b)

    # ---------------- read-vector assembly (all batches) ----------------
    # rvt_full[(c',d), b] = r1 - e*r2 + a*swr  on both partition halves,
    # then one transpose and a free-axis fold of the halves.
    ps_swb = ps_misc.tile([128, 144], F32, tag="pm")
    nc.tensor.matmul(ps_swb[:, 0:BL], ones_sb[0:1, :], swr_sb,
                     start=True, stop=True)
    rvt = work.tile([128, BL], F32, tag="rvt", name="rvt")
    nc.vector.tensor_mul(rvt, e2_sb, r12_all[:, :, 1])   # e * r2
    nc.vector.tensor_sub(rvt, r12_all[:, :, 0], rvt)     # r1 - e*r2
    m3 = work.tile([128, BL], F32, tag="m3", name="m3")
    nc.vector.tensor_mul(m3, a2_sb, ps_swb[:, 0:BL])     # a * sum(wr*ww)
    nc.vector.tensor_add(rvt, rvt, m3)
    ps_rvo = ps_misc.tile([128, 144], F32, tag="pm")
    nc.tensor.transpose(ps_rvo[0:BL, 0:128], rvt, ident_sb)
    rvo_sb = work.tile([BL, 128], F32, tag="rvo_sb", name="rvo_sb")
    nc.vector.tensor_copy(rvo_sb, ps_rvo[0:BL, 0:128])
    nc.vector.tensor_add(out_sb[:, C:C + D], rvo_sb[:, 0:64],
                         rvo_sb[:, 64:128])

    nc.sync.dma_start(out=out_d[:], in_=out_sb)
    ctx.close()


# ---------------------------------------------------------------------------
# host-side driver
# ---------------------------------------------------------------------------
_NC = None


def _get_module():
    global _NC
    if _NC is None:
        _NC = _build_module()
    return _NC


def _consts():
    ident = np.eye(128, dtype=np.float32)
    onest = np.ones((128, 128), np.float32)
    permu = np.zeros((128, 128), np.float32)
    permd = np.zeros((128, 128), np.float32)
    for m in range(128):
        permu[(m + 1) % 128, m] = 1.0
        permd[(m - 1) % 128, m] = 1.0
    sel = np.zeros((32, NQ * 128), np.float32)
    for q in range(NQ):
        sel[q, q * 128:(q + 1) * 128] = 1.0
    return ident, onest, permu, permd, sel


def kernel(**inputs):
    from concourse.bass_utils import run_bass_kernel_spmd

    nc = _get_module()
    f = lambda k: np.ascontiguousarray(np.asarray(inputs[k], np.float32))

    whead = np.concatenate([
        f("Wk_r"), f("Wb_r"), f("Wg_r"), f("Ws_r"), f("Wgam_r"),
        f("Wk_w"), f("Wb_w"), f("Wg_w"), f("Ws_w"), f("Wgam_w"),
        f("We_w"), f("Wa_w")], axis=1)
    bhead = np.concatenate([
        f("bk_r"), f("bb_r"), f("bg_r"), f("bs_r"), f("bgam_r"),
        f("bk_w"), f("bb_w"), f("bg_w"), f("bs_w"), f("bgam_w"),
        f("be_w"), f("ba_w")])
    ident, onest, permu, permd, sel = _consts()

    mem = f("prev_memory")
    x = f("x")
    rv = f("prev_read_vector")
    prw = f("prev_read_weights")
    pww = f("prev_write_weights")
    shared = dict(wctrl=f("W_ctrl"), bctrl=f("b_ctrl"), whead=whead,
                  bhead=bhead, ident=ident, onest=onest, permu=permu,
                  permd=permd, sel=sel)
    in_maps = []
    for c in range(NCORES):
        sl = slice(c * BL, (c + 1) * BL)
        in_maps.append(dict(
            mem=np.ascontiguousarray(mem[sl]),
            x=np.ascontiguousarray(x[sl]),
            rv=np.ascontiguousarray(rv[sl]),
            prw=np.ascontiguousarray(prw[sl]),
            pww=np.ascontiguousarray(pww[sl]),
            **shared))
    res = run_bass_kernel_spmd(nc, in_maps, list(range(NCORES)))
    return np.concatenate([res.results[c]["out"] for c in range(NCORES)],
                          axis=0).astype(np.float32)


# revision 35
# speedup vs baseline: 1.0230x; 1.0195x over previous
"""NTM cell kernel for Trainium2 (8 NeuronCores, batch-parallel).

Strategy (per core, 8 batches):
  - prev_memory slice is cast-loaded f32->bf16 into SBUF (row-major M16).
  - The on-chip transpose to T16 runs on the TENSOR engine (128x128 bf16
    transposes into PSUM, ~1 cyc/row) instead of the DMA xbar, freeing the
    DMA pool for the HBM load; PSUM->SBUF cast copies alternate between
    the vector and scalar engines, elementwise squares (T2) between
    gpsimd and vector.
  - All O(N*D) reductions run on the tensor engine:
      * content dots + sum-of-squares streams over T16 / T2 (stationary
        rides the FWL weight path at ~0.5 cyc/col)
      * read-vector contraction with the memory chunk-pair as the
        128-col stationary and [w_r, w_r*w_w] as a 4-wide moving; the
        two chunk-halves land on partition halves and are folded after
        the final output transpose.
  - new_memory is never materialized; its dot/norm/read contributions are
    expanded algebraically in terms of streams over the ORIGINAL memory.
  - Addressing chains (softmax/gate/shift/sharpen) run on DVE/ACT/GPSIMD
    in a [128 x 64] layout (n = p*64 + c), with per-group buffers so the
    two batch-groups' chains pipeline instead of serializing.
  - Only one ACT table set is used (exp/ln); sqrt/sigmoid/tanh/softplus
    are rewritten via exp/ln so no table reloads occur.
"""

import sys

sys.path.insert(0, "/opt/trn_rl_repo")

import numpy as np

import concourse.bass as bass
import concourse.tile as tile
from concourse import mybir

F32 = mybir.dt.float32
BF16 = mybir.dt.bfloat16
AF = mybir.ActivationFunctionType
OP = mybir.AluOpType

B, N, D, C, IN, S = 64, 8192, 64, 256, 128, 3
NCORES = 8
BL = B // NCORES          # batches per core
P = 128                   # partitions
CH = N // P               # 64 chunks per batch (n = p*64 + c)
NPAIR = CH // 2           # 32 transposed tiles per batch
EPS = 1e-8

# whead column map
KR0, KR1 = 0, 64
BR, GR = 64, 65
SR0, SR1 = 66, 69
GAMR = 69
KW0, KW1 = 70, 134
BW, GW = 134, 135
SW0, SW1 = 136, 139
GAMW = 139
E0, E1 = 140, 204
A0, A1 = 204, 268
NHEAD = 268

# scalar table rows (S8 cols -> SC rows -> BC blocks of 8)
Q_BET_W, Q_G_W, Q_OMG_W, Q_SW0, Q_SW1, Q_SW2, Q_GAM_W, Q_NK2_W = range(8)
Q_BET_R, Q_G_R, Q_OMG_R, Q_SR0, Q_SR1, Q_SR2, Q_GAM_R, Q_NK2_R = range(8, 16)
Q_AKR, Q_AA = 16, 17
NQ = 18

GRP = 4  # batches per pipeline group

# ---------------------------------------------------------------------------
# workaround: the deployed walrus accepts only ONE sem-wait per instruction.
# After TileContext exits, hoist extra waits onto injected single-wait nops
# (drains on the SP engine, ENGINE_NOPs elsewhere).
# ---------------------------------------------------------------------------
import concourse.tile as tile_mod


def _split_multi_waits(nc):
    for f in nc.m.functions:
        for b in f.blocks:
            insts = b.instructions
            i = 0
            while i < len(insts):
                ins = insts[i]
                si = getattr(ins, "sync_info", None)
                if si is None or len(si.on_wait) <= 1:
                    i += 1
                    continue
                waits = list(si.on_wait)
                ins.sync_info = mybir.SyncInfo(
                    on_wait=[waits[-1]], on_update=list(si.on_update)
                )
                eng = nc.engines[ins.engine]
                new_nops = []
                for w in waits[:-1]:
                    nop = eng.isa(
                        nc.isa.Opcode.NEURON_ISA_TPB_OPCODE_NOP, {}
                    ).ins
                    nop.sync_info = mybir.SyncInfo(on_wait=[w], on_update=[])
                    new_nops.append(nop)
                for nop in new_nops:
                    for bb2 in f.blocks:
                        try:
                            bb2.instructions.remove(nop)
                            break
                        except ValueError:
                            pass
                for k, nop in enumerate(new_nops):
                    insts.insert(i + k, nop)
                i += len(new_nops) + 1


_orig_exit = tile_mod.TileContext.__exit__


def _patched_exit(self, *a, **k):
    import os
    r = _orig_exit(self, *a, **k)
    if not os.environ.get("NTM_NO_WAITFIX"):
        _split_multi_waits(self.nc)
    return r


if not getattr(tile_mod.TileContext, "_waitfix_patched", False):
    tile_mod.TileContext.__exit__ = _patched_exit
    tile_mod.TileContext._waitfix_patched = True


# ---------------------------------------------------------------------------
# kernel body
# ---------------------------------------------------------------------------

def _build_module():
    nc = bass.Bass()

    mem = nc.dram_tensor("mem", [BL, N, D], F32, kind="ExternalInput")
    x_in = nc.dram_tensor("x", [BL, IN], F32, kind="ExternalInput")
    rv_in = nc.dram_tensor("rv", [BL, D], F32, kind="ExternalInput")
    prw_in = nc.dram_tensor("prw", [BL, N], F32, kind="ExternalInput")
    pww_in = nc.dram_tensor("pww", [BL, N], F32, kind="ExternalInput")
    wctrl = nc.dram_tensor("wctrl", [IN + D, C], F32, kind="ExternalInput")
    bctrl = nc.dram_tensor("bctrl", [C], F32, kind="ExternalInput")
    whead = nc.dram_tensor("whead", [C, NHEAD], F32, kind="ExternalInput")
    bhead = nc.dram_tensor("bhead", [NHEAD], F32, kind="ExternalInput")
    ident = nc.dram_tensor("ident", [128, 128], F32, kind="ExternalInput")
    onest = nc.dram_tensor("onest", [128, 128], F32, kind="ExternalInput")
    permu = nc.dram_tensor("permu", [128, 128], F32, kind="ExternalInput")
    permd = nc.dram_tensor("permd", [128, 128], F32, kind="ExternalInput")
    seldr = nc.dram_tensor("sel", [32, NQ * 128], F32, kind="ExternalInput")
    out_d = nc.dram_tensor("out", [BL, C + D], F32, kind="ExternalOutput")

    with tile.TileContext(nc) as tc:
        _emit(nc, tc, mem, x_in, rv_in, prw_in, pww_in, wctrl, bctrl, whead,
              bhead, ident, onest, permu, permd, seldr, out_d)
    return nc


def _emit(nc, tc, mem, x_in, rv_in, prw_in, pww_in, wctrl, bctrl, whead,
          bhead, ident, onest, permu, permd, seldr, out_d):
    from contextlib import ExitStack

    ctx = ExitStack()
    ctx.enter_context(nc.allow_low_precision(
        reason="bf16 chain intermediates; rel-err budget 2e-2"))
    big = ctx.enter_context(tc.tile_pool(name="big", bufs=1))
    cons = ctx.enter_context(tc.tile_pool(name="cons", bufs=1))
    work = ctx.enter_context(tc.tile_pool(name="work", bufs=1))
    t16p = ctx.enter_context(tc.tile_pool(name="t16p", bufs=3))
    t2p = ctx.enter_context(tc.tile_pool(name="t2p", bufs=2))
    qallp = ctx.enter_context(tc.tile_pool(name="qallp", bufs=2))
    ps_tr = ctx.enter_context(tc.tile_pool(name="ps_tr", bufs=2, space="PSUM"))
    ps_stream = ctx.enter_context(tc.tile_pool(name="ps_stream", bufs=2, space="PSUM"))
    ps_misc = ctx.enter_context(tc.tile_pool(name="ps_misc", bufs=3, space="PSUM"))
    ps_rvp = ctx.enter_context(tc.tile_pool(name="ps_rvp", bufs=1, space="PSUM"))

    # ---------------- big memory load: issue FIRST ----------------
    # batch 0 on sync so its descriptors hit the rings first; the rest
    # sequentially behind it on gpsimd (per-ring FIFO keeps completion
    # roughly batch-ordered).
    # bf16 const cast-loads go FIRST on the gpsimd SW queue (tiny), then
    # the 8 per-batch memory cast-loads (f32->bf16 in the DGE) behind them.
    m16s = [big.tile([P, CH, D], BF16, tag=f"m16_{b}", name=f"m16_{b}")
            for b in range(BL)]

    identb_sb = cons.tile([128, 128], BF16, tag="identb")
    nc.gpsimd.dma_start(out=identb_sb, in_=ident[:])
    wh0 = cons.tile([128, NHEAD], BF16, tag="wh0")
    nc.gpsimd.dma_start(out=wh0, in_=whead[0:128, :])
    wh1 = cons.tile([128, NHEAD], BF16, tag="wh1")
    nc.gpsimd.dma_start(out=wh1, in_=whead[128:256, :])
    bh_sb = cons.tile([1, NHEAD], BF16, tag="bh")
    nc.gpsimd.dma_start(out=bh_sb, in_=bhead.rearrange("(o n) -> o n", o=1))
    selb_sb = cons.tile([32, NQ * 128], BF16, tag="selb")
    nc.gpsimd.dma_start(out=selb_sb, in_=seldr[:])
    onesb_sb = cons.tile([1, 128], BF16, tag="onesb")
    nc.gpsimd.dma_start(out=onesb_sb, in_=onest[0:1, :])
    permub_sb = cons.tile([128, 128], BF16, tag="permub")
    nc.gpsimd.dma_start(out=permub_sb, in_=permu[:])
    permdb_sb = cons.tile([128, 128], BF16, tag="permdb")
    nc.gpsimd.dma_start(out=permdb_sb, in_=permd[:])
    for b in range(BL):
        nc.gpsimd.dma_start(
            out=m16s[b], in_=mem[b].rearrange("(p c) d -> p c d", p=128)
        )

    wc0 = cons.tile([128, C], F32, tag="wc0")
    nc.sync.dma_start(out=wc0, in_=wctrl[0:128, :])
    wc1 = cons.tile([64, C], F32, tag="wc1")
    nc.sync.dma_start(out=wc1, in_=wctrl[128:192, :])
    bc_sb = cons.tile([128, 2], F32, tag="bc")
    nc.sync.dma_start(out=bc_sb, in_=bctrl.rearrange("(j p) -> p j", p=128))
    xt_in = cons.tile([BL, IN], F32, tag="xt_in")
    nc.sync.dma_start(out=xt_in, in_=x_in[:])
    rv_sb = cons.tile([BL, D], F32, tag="rv_sb")
    nc.sync.dma_start(out=rv_sb, in_=rv_in[:])
    ident_sb = cons.tile([128, 128], F32, tag="ident")
    nc.sync.dma_start(out=ident_sb, in_=ident[:])

    ones_sb = cons.tile([128, 128], F32, tag="ones")
    nc.scalar.dma_start(out=ones_sb, in_=onest[:])
    permu_sb = cons.tile([128, 128], F32, tag="permu")
    nc.scalar.dma_start(out=permu_sb, in_=permu[:])
    permd_sb = cons.tile([128, 128], F32, tag="permd")
    nc.scalar.dma_start(out=permd_sb, in_=permd[:])
    pw_w = cons.tile([128, BL, CH], F32, tag="pw_w")
    nc.scalar.dma_start(out=pw_w, in_=pww_in.rearrange("b (p c) -> p b c", p=128))
    pw_r = cons.tile([128, BL, CH], F32, tag="pw_r")
    nc.scalar.dma_start(out=pw_r, in_=prw_in.rearrange("b (p c) -> p b c", p=128))


    # ---------------- controller: hT = relu(W_ctrl^T @ ctrl_in^T + b) -------
    ps_xt = ps_misc.tile([128, 144], F32, tag="pm")
    nc.tensor.transpose(ps_xt[:, 0:BL], xt_in, ident_sb[0:BL, 0:BL])
    xT = work.tile([128, BL], F32, tag="xT")
    nc.vector.tensor_copy(xT, ps_xt[:, 0:BL])
    ps_rt = ps_misc.tile([128, 144], F32, tag="pm")
    nc.tensor.transpose(ps_rt[0:D, 0:BL], rv_sb, ident_sb[0:BL, 0:BL])
    rvT = work.tile([64, BL], F32, tag="rvT")
    nc.vector.tensor_copy(rvT, ps_rt[0:D, 0:BL])

    hT_sb = []
    for j in range(2):
        ps_h = ps_misc.tile([128, 144], F32, tag="pm")
        nc.tensor.matmul(ps_h[:, 0:BL], wc0[:, j * 128:(j + 1) * 128], xT,
                         start=True, stop=False)
        nc.tensor.matmul(ps_h[:, 0:BL], wc1[:, j * 128:(j + 1) * 128], rvT,
                         start=False, stop=True)
        h_j = work.tile([128, BL], F32, tag=f"hT{j}")
        nc.scalar.activation(h_j, ps_h[:, 0:BL], AF.Relu,
                             bias=bc_sb[:, j:j + 1], scale=1.0)
        hT_sb.append(h_j)

    # ---------------- head params P = h @ Whead + bhead (bf16) ----------
    hT_b = []
    for j in range(2):
        hb = work.tile([128, BL], BF16, tag=f"hTb{j}", name=f"hTb{j}")
        nc.vector.tensor_copy(hb, hT_sb[j])
        hT_b.append(hb)
    ps_p = ps_misc.tile([BL, 512], F32, tag="pm")
    nc.tensor.matmul(ps_p[:, 0:NHEAD], hT_b[0], wh0, start=True, stop=False)
    nc.tensor.matmul(ps_p[:, 0:NHEAD], hT_b[1], wh1, start=False, stop=False)
    nc.tensor.matmul(ps_p[:, 0:NHEAD], onesb_sb[0:1, 0:BL], bh_sb,
                     start=False, stop=True)
    p_sb = work.tile([BL, NHEAD], F32, tag="p_sb")
    nc.vector.tensor_copy(p_sb, ps_p[:, 0:NHEAD])

    # ---------------- VA: per-batch d-vectors [BL, 8*64] ----------------
    # vec order: 0 k_w, 1 k_r, 2 e*k_r, 3 a, 4 a*e, 5 ones, 6 e, 7 e^2
    va = work.tile([BL, 512], F32, tag="va")
    nc.vector.tensor_copy(va[:, 0:64], p_sb[:, KW0:KW1])
    nc.vector.tensor_copy(va[:, 64:128], p_sb[:, KR0:KR1])

    def _sigmoid(dst, src):  # dst = 1/(1+exp(-src))
        nc.scalar.activation(dst, src, AF.Exp, scale=-1.0)
        nc.vector.tensor_scalar_add(dst, dst, 1.0)
        nc.vector.reciprocal(dst, dst)

    # e = sigmoid(P_e) -> va[:, 384:448]
    _sigmoid(va[:, 384:448], p_sb[:, E0:E1])
    # a = tanh(P_a) = 1 - 2/(exp(2x)+1) -> va[:, 192:256]
    nc.scalar.activation(va[:, 192:256], p_sb[:, A0:A1], AF.Exp, scale=2.0)
    nc.vector.tensor_scalar_add(va[:, 192:256], va[:, 192:256], 1.0)
    nc.vector.reciprocal(va[:, 192:256], va[:, 192:256])
    nc.vector.tensor_scalar(va[:, 192:256], va[:, 192:256], -2.0, 1.0,
                            op0=OP.mult, op1=OP.add)
    # beta' = softplus(P_beta) * rsqrt(||k||^2): folded into the stream
    # vectors so the chain's beta-mul and nk2-normalization disappear.
    tmp64p = work.tile([BL, 64], F32, tag="tmp64p")
    bp = work.tile([BL, 4], F32, tag="bp")
    nc.vector.tensor_mul(tmp64p, va[:, 0:64], va[:, 0:64])
    nc.vector.reduce_sum(bp[:, 0:1], tmp64p, axis=mybir.AxisListType.X)
    nc.vector.tensor_mul(tmp64p, va[:, 64:128], va[:, 64:128])
    nc.vector.reduce_sum(bp[:, 1:2], tmp64p, axis=mybir.AxisListType.X)
    nc.scalar.activation(bp[:, 0:2], bp[:, 0:2], AF.Ln)
    nc.scalar.activation(bp[:, 0:2], bp[:, 0:2], AF.Exp, scale=-0.5)
    nc.scalar.activation(bp[:, 2:3], p_sb[:, BW:BW + 1], AF.Exp)
    nc.scalar.activation(bp[:, 3:4], p_sb[:, BR:BR + 1], AF.Exp)
    nc.vector.tensor_scalar_add(bp[:, 2:4], bp[:, 2:4], 1.0)
    nc.scalar.activation(bp[:, 2:4], bp[:, 2:4], AF.Ln)
    nc.vector.tensor_mul(bp[:, 2:3], bp[:, 2:3], bp[:, 0:1])
    nc.vector.tensor_mul(bp[:, 3:4], bp[:, 3:4], bp[:, 1:2])
    nc.vector.tensor_scalar(va[:, 0:64], va[:, 0:64], bp[:, 2:3], None,
                            op0=OP.mult)
    nc.vector.tensor_scalar(va[:, 64:128], va[:, 64:128], bp[:, 3:4], None,
                            op0=OP.mult)
    # e*k_r (scaled), a*e, ones, e^2
    nc.vector.tensor_mul(va[:, 128:192], va[:, 384:448], va[:, 64:128])
    nc.vector.tensor_mul(va[:, 256:320], va[:, 192:256], va[:, 384:448])
    nc.vector.memset(va[:, 320:384], 1.0)
    nc.vector.tensor_mul(va[:, 448:512], va[:, 384:448], va[:, 384:448])

    # ---------------- VTD: transposed vectors with zero-halves --------------
    # VTD[p, half, vec, b]; half 0: rows 0-63 hold vec, rows 64-127 zero.
    vtd = work.tile([128, 2, 8, BL], BF16, tag="vtd")
    nc.vector.memset(vtd, 0.0)
    vapad = work.tile([BL, 8, 128], F32, tag="vapad")
    nc.vector.memset(vapad, 0.0)
    for v in range(8):
        nc.vector.tensor_copy(vapad[:, v, 64:128], va[:, v * 64:(v + 1) * 64])
    ps_top = ps_misc.tile([128, 144], F32, tag="pm")
    ps_bot = ps_misc.tile([128, 144], F32, tag="pm")
    for v in range(8):
        nc.tensor.transpose(ps_top[0:64, v * BL:(v + 1) * BL],
                            va[:, v * 64:(v + 1) * 64],
                            ident_sb[0:BL, 0:BL])
        nc.tensor.transpose(ps_bot[:, v * BL:(v + 1) * BL],
                            vapad[:, v, :], ident_sb[0:BL, 0:BL])
    nc.vector.tensor_copy(
        vtd[0:64].rearrange("p h v b -> p (h v b)")[:, 0:64],
        ps_top[0:64, 0:64])
    nc.vector.tensor_copy(
        vtd[64:128].rearrange("p h v b -> p (h v b)")[:, 64:128],
        ps_bot[64:128, 0:64])
    # e/a duplicated across both partition halves for the rv assembly:
    # ea_dup[b, 0:64] = e, [64:128] = e  (same for a) -> transpose -> [128, BL]
    ea_dup = work.tile([BL, 2, 128], F32, tag="ea_dup")
    nc.vector.tensor_copy(ea_dup[:, 0, 0:64], va[:, 384:448])
    nc.vector.tensor_copy(ea_dup[:, 0, 64:128], va[:, 384:448])
    nc.vector.tensor_copy(ea_dup[:, 1, 0:64], va[:, 192:256])
    nc.vector.tensor_copy(ea_dup[:, 1, 64:128], va[:, 192:256])
    ps_ea = ps_misc.tile([128, 144], F32, tag="pm")
    nc.tensor.transpose(ps_ea[:, 0:BL], ea_dup[:, 0, :], ident_sb[0:BL, 0:BL])
    nc.tensor.transpose(ps_ea[:, BL:2 * BL], ea_dup[:, 1, :],
                        ident_sb[0:BL, 0:BL])
    e2_sb = work.tile([128, BL], F32, tag="e2_sb")
    nc.vector.tensor_copy(e2_sb, ps_ea[:, 0:BL])
    a2_sb = work.tile([128, BL], F32, tag="a2_sb")
    nc.vector.tensor_copy(a2_sb, ps_ea[:, BL:2 * BL])

    # ---------------- per-batch scalars S8 [BL, 32] ----------------
    s8 = work.tile([BL, 32], F32, tag="s8")
    nc.vector.memset(s8, 0.0)
    tmp64 = work.tile([BL, 64], F32, tag="tmp64")

    def _softplus(dst, src):  # ln(1 + exp(src))
        nc.scalar.activation(dst, src, AF.Exp)
        nc.vector.tensor_scalar_add(dst, dst, 1.0)
        nc.scalar.activation(dst, dst, AF.Ln)

    def _softmax3(dst, src):
        ex3 = work.tile([BL, 3], F32, tag="ex3")
        nc.scalar.activation(ex3, src, AF.Exp)
        sm = work.tile([BL, 1], F32, tag="sm3")
        nc.vector.reduce_sum(sm, ex3, axis=mybir.AxisListType.X)
        nc.vector.reciprocal(sm, sm)
        nc.vector.tensor_scalar(dst, ex3, sm, None, op0=OP.mult)

    _softplus(s8[:, Q_BET_W:Q_BET_W + 1], p_sb[:, BW:BW + 1])
    _sigmoid(s8[:, Q_G_W:Q_G_W + 1], p_sb[:, GW:GW + 1])
    nc.vector.tensor_scalar(s8[:, Q_OMG_W:Q_OMG_W + 1],
                            s8[:, Q_G_W:Q_G_W + 1], -1.0, 1.0,
                            op0=OP.mult, op1=OP.add)
    _softmax3(s8[:, Q_SW0:Q_SW0 + 3], p_sb[:, SW0:SW1])
    _softplus(s8[:, Q_GAM_W:Q_GAM_W + 1], p_sb[:, GAMW:GAMW + 1])
    nc.vector.tensor_scalar_add(s8[:, Q_GAM_W:Q_GAM_W + 1],
                                s8[:, Q_GAM_W:Q_GAM_W + 1], 1.0)
    nc.vector.tensor_mul(tmp64, va[:, 0:64], va[:, 0:64])
    nc.vector.reduce_sum(s8[:, Q_NK2_W:Q_NK2_W + 1], tmp64,
                         axis=mybir.AxisListType.X)

    _softplus(s8[:, Q_BET_R:Q_BET_R + 1], p_sb[:, BR:BR + 1])
    _sigmoid(s8[:, Q_G_R:Q_G_R + 1], p_sb[:, GR:GR + 1])
    nc.vector.tensor_scalar(s8[:, Q_OMG_R:Q_OMG_R + 1],
                            s8[:, Q_G_R:Q_G_R + 1], -1.0, 1.0,
                            op0=OP.mult, op1=OP.add)
    _softmax3(s8[:, Q_SR0:Q_SR0 + 3], p_sb[:, SR0:SR1])
    _softplus(s8[:, Q_GAM_R:Q_GAM_R + 1], p_sb[:, GAMR:GAMR + 1])
    nc.vector.tensor_scalar_add(s8[:, Q_GAM_R:Q_GAM_R + 1],
                                s8[:, Q_GAM_R:Q_GAM_R + 1], 1.0)
    nc.vector.tensor_mul(tmp64, va[:, 64:128], va[:, 64:128])
    nc.vector.reduce_sum(s8[:, Q_NK2_R:Q_NK2_R + 1], tmp64,
                         axis=mybir.AxisListType.X)

    nc.vector.tensor_mul(tmp64, va[:, 192:256], va[:, 64:128])
    nc.vector.reduce_sum(s8[:, Q_AKR:Q_AKR + 1], tmp64,
                         axis=mybir.AxisListType.X)
    nc.vector.tensor_mul(tmp64, va[:, 192:256], va[:, 192:256])
    nc.vector.reduce_sum(s8[:, Q_AA:Q_AA + 1], tmp64,
                         axis=mybir.AxisListType.X)

    # transpose S8 -> SC [32, BL] and broadcast -> BC [128, NQ*8]
    ps_sc = ps_misc.tile([128, 144], F32, tag="pm")
    nc.tensor.transpose(ps_sc[0:32, 0:BL], s8, ident_sb[0:BL, 0:BL])
    sc_sb = work.tile([32, BL], BF16, tag="sc_sb")
    nc.vector.tensor_copy(sc_sb, ps_sc[0:32, 0:BL])
    ps_bc = ps_misc.tile([128, 144], F32, tag="pm")
    for q in range(NQ):
        nc.tensor.matmul(ps_bc[:, q * BL:(q + 1) * BL],
                         selb_sb[:, q * 128:(q + 1) * 128], sc_sb,
                         start=True, stop=True)
    bc_all = work.tile([128, NQ * BL], BF16, tag="bc_all")
    nc.vector.tensor_copy(bc_all, ps_bc[:, 0:NQ * BL])
    bc_f32 = work.tile([128, NQ * BL], F32, tag="bc_f32")
    nc.vector.tensor_copy(bc_f32, ps_bc[:, 0:NQ * BL])

    def BCF(q, b):
        return bc_f32[:, q * BL + b:q * BL + b + 1]

    def BC(q, b):
        return bc_all[:, q * BL + b:q * BL + b + 1]

    # ---------------- output staging ----------------
    out_sb = work.tile([BL, C + D], F32, tag="out_sb")
    ps_ho = ps_misc.tile([128, 144], F32, tag="pm")
    nc.tensor.transpose(ps_ho[0:BL, 0:128], hT_sb[0], ident_sb)
    nc.vector.tensor_copy(out_sb[:, 0:128], ps_ho[0:BL, 0:128])
    ps_ho2 = ps_misc.tile([128, 144], F32, tag="pm")
    nc.tensor.transpose(ps_ho2[0:BL, 0:128], hT_sb[1], ident_sb)
    nc.vector.tensor_copy(out_sb[:, 128:256], ps_ho2[0:BL, 0:128])

    swr_sb = work.tile([1, BL], F32, tag="swr_sb")
    r12_all = work.tile([128, BL, 2], F32, tag="r12_all")

    # ---------------- helpers for grouped heavy phase ----------------
    def scb3(q, gs):
        base = bc_all[:, q * BL + gs:q * BL + gs + GRP]
        return bass.AP(tensor=base.tensor, offset=base.offset,
                       ap=[base.ap[0], base.ap[1], [0, CH]])

    def scb3n(q, gs, n):
        base = bc_all[:, q * BL + gs:q * BL + gs + GRP]
        return bass.AP(tensor=base.tensor, offset=base.offset,
                       ap=[base.ap[0], base.ap[1], [0, n]])

    def bc3(t8):
        base = t8[:, :]
        return bass.AP(tensor=base.tensor, offset=base.offset,
                       ap=[base.ap[0], base.ap[1], [0, CH]])

    def ctile(tag, gi):
        tg = f"{tag}_g{gi}"
        return work.tile([P, GRP, CH], BF16, tag=tg, name=tg)

    def gtile(tag, gi, dt=F32):
        tg = f"{tag}_g{gi}"
        return work.tile([128, GRP], dt, tag=tg, name=tg)

    def psum_colsum_bcast(cs8, gi, eps=None, tag="tot"):
        # one matmul with a full ones stationary both sums over partitions
        # and broadcasts the per-batch total to every output partition
        ps_t = ps_misc.tile([128, 144], F32, tag="pm")
        nc.tensor.matmul(ps_t[:, 0:GRP], ones_sb, cs8, start=True, stop=True)
        rt = gtile(tag + "_rt", gi)
        if eps is not None:
            nc.vector.tensor_scalar_add(rt, ps_t[:, 0:GRP], eps)
            nc.vector.reciprocal(rt, rt)
        else:
            nc.vector.reciprocal(rt, ps_t[:, 0:GRP])
        return rt

    def w_chain_all(dk_v, ssm_v, pw_all, qo, gs, gi, dst):
        bet, g_, omg, s0, s1, s2, gam, nk2 = (qo + i for i in range(8))
        v = ctile("wc_v", gi)
        nc.scalar.activation(v, ssm_v, AF.Ln)
        inv = ctile("wc_inv", gi)
        nc.scalar.activation(inv, v, AF.Exp, scale=-0.5)
        bsim = ctile("wc_bsim", gi)
        nc.vector.tensor_mul(bsim, dk_v, inv)
        # exp + per-partition row-sum fused on ACT, one per batch
        ex = ctile("wc_ex", gi)
        cs = gtile("wc_cs", gi, F32)
        nc.scalar.activation(ex, bsim, AF.Exp)
        nc.vector.reduce_sum(cs, ex, axis=mybir.AxisListType.X)
        # sharpening is scale-invariant, so fold the content-softmax
        # denominator T into the interpolation instead of normalizing:
        # ws' = g*ex + T*(1-g)*pw  (T broadcast by the colsum matmul)
        ps_T = ps_misc.tile([128, 144], F32, tag="pm")
        nc.tensor.matmul(ps_T[:, 0:GRP], ones_sb, cs, start=True, stop=True)
        omgT = gtile("wc_omgT", gi)
        nc.vector.tensor_mul(omgT, ps_T[:, 0:GRP],
                             bc_all[:, omg * BL + gs:omg * BL + gs + GRP])
        t9 = ctile("wc_t9", gi)
        nc.vector.tensor_mul(t9, pw_all, bc3(omgT))
        wg = ctile("wc_wg", gi)
        for j in range(GRP):
            nc.scalar.activation(wg[:, j], ex[:, j], AF.Copy,
                                 scale=BCF(g_, gs + j))
        nc.vector.tensor_add(wg, wg, t9)
        # circular shift: body via shifted APs, boundary cols via perm matmuls
        ps_sh = ps_misc.tile([128, 144], F32, tag="pm")
        nc.tensor.matmul(ps_sh[:, 0:GRP], permub_sb, wg[:, :, 0],
                         start=True, stop=True)
        nc.tensor.matmul(ps_sh[:, GRP:2 * GRP], permdb_sb, wg[:, :, CH - 1],
                         start=True, stop=True)
        ws = ctile("wc_ws", gi)
        for j in range(GRP):
            nc.scalar.activation(ws[:, j], wg[:, j], AF.Copy,
                                 scale=BCF(s1, gs + j))
        tA = ctile("wc_tA", gi)
        nc.vector.tensor_mul(tA[:, :, 0:CH - 1], wg[:, :, 1:CH],
                             scb3n(s0, gs, CH - 1))
        nc.vector.tensor_add(ws[:, :, 0:CH - 1], ws[:, :, 0:CH - 1],
                             tA[:, :, 0:CH - 1])
        nc.vector.tensor_mul(tA[:, :, 1:CH], wg[:, :, 0:CH - 1],
                             scb3n(s2, gs, CH - 1))
        nc.vector.tensor_add(ws[:, :, 1:CH], ws[:, :, 1:CH],
                             tA[:, :, 1:CH])
        bnd = work.tile([128, 2 * GRP], F32, tag=f"wc_bnd_g{gi}",
                        name=f"wc_bnd_g{gi}")
        nc.vector.tensor_mul(bnd[:, 0:GRP], ps_sh[:, 0:GRP],
                             bc_all[:, s0 * BL + gs:s0 * BL + gs + GRP])
        nc.vector.tensor_mul(bnd[:, GRP:2 * GRP], ps_sh[:, GRP:2 * GRP],
                             bc_all[:, s2 * BL + gs:s2 * BL + gs + GRP])
        nc.vector.tensor_add(ws[:, :, CH - 1], ws[:, :, CH - 1],
                             bnd[:, 0:GRP])
        nc.vector.tensor_add(ws[:, :, 0], ws[:, :, 0], bnd[:, GRP:2 * GRP])
        # sharpening: wp = exp(gam * ln(ws)) with fused row-sums
        lg = ctile("wc_lg", gi)
        nc.scalar.activation(lg, ws, AF.Ln)
        wp = ctile("wc_wp", gi)
        cs2 = gtile("wc_cs2", gi, F32)
        for j in range(GRP):
            nc.scalar.activation(wp[:, j], lg[:, j], AF.Exp,
                                 scale=BCF(gam, gs + j))
        nc.vector.reduce_sum(cs2, wp, axis=mybir.AxisListType.X)
        rt2 = psum_colsum_bcast(cs2, gi, eps=EPS, tag="wc_t2")
        for j in range(GRP):
            nc.scalar.activation(dst[:, j], wp[:, j], AF.Copy,
                                 scale=rt2[:, j:j + 1])

    # ---------------- per-batch heavy stream ----------------
    qalls = {}

    def emit_batch(b):
        gi, bb = b // GRP, b % GRP
        if bb == 0:
            qalls[gi] = qallp.tile([P, GRP, 512], BF16, tag="qall",
                                   name="qall")
        qall = qalls[gi]
        t16b = t16p.tile([P, NPAIR, 128], BF16, tag="t16b", name="t16b")
        t2b = t2p.tile([P, NPAIR, 128], BF16, tag="t2b", name="t2b")
        m16f = m16s[b].rearrange("p c d -> p (c d)")
        if b == BL - 1:
            # the DMA pool is idle once the load drains: xbar-transpose the
            # last batch to take its transposes+copies off the PE/ACT path
            nc.sync.dma_start_transpose(t16b[:, 0:16], m16f[:, 0:2048])
            nc.scalar.dma_start_transpose(t16b[:, 16:32], m16f[:, 2048:4096])
            big0 = t16b[:, 0:16].rearrange("p a q -> p (a q)")
            big0d = t2b[:, 0:16].rearrange("p a q -> p (a q)")
            nc.gpsimd.tensor_mul(big0d, big0, big0)
            big1 = t16b[:, 16:32].rearrange("p a q -> p (a q)")
            big1d = t2b[:, 16:32].rearrange("p a q -> p (a q)")
            nc.vector.tensor_mul(big1d, big1, big1)
            _emit_streams(b, bb, qall, t16b, t2b)
            return
        # transposes in quads sharing one PSUM bank; copies alternate
        # vector/scalar, squares alternate gpsimd/vector
        for q in range(4):
            ps_t = ps_tr.tile([128, 1024], BF16, tag="ps_t")
            for k in range(8):
                cp = q * 8 + k
                nc.tensor.transpose(ps_t[:, k * 128:(k + 1) * 128],
                                    m16f[:, cp * 128:(cp + 1) * 128],
                                    identb_sb)
            t16v = t16b[:, q * 8:(q + 1) * 8].rearrange("p a q -> p (a q)")
            t2v = t2b[:, q * 8:(q + 1) * 8].rearrange("p a q -> p (a q)")
            if q == 2:
                nc.vector.tensor_copy(t16v, ps_t)
            else:
                nc.scalar.activation(t16v, ps_t, AF.Copy)
            if q == 1:
                big2 = t16b[:, 0:16].rearrange("p a q -> p (a q)")
                big2d = t2b[:, 0:16].rearrange("p a q -> p (a q)")
                nc.gpsimd.tensor_mul(big2d, big2, big2)
            elif q >= 2:
                nc.vector.tensor_mul(t2v, t16v, t16v)
        _emit_streams(b, bb, qall, t16b, t2b)

    def _emit_streams(b, bb, qall, t16b, t2b):
        pb = ps_stream.tile([128, 512], F32, tag="pb")
        rhs_m = vtd[:, :, 0:5, b].rearrange("p h v -> p v h")
        rhs_s = vtd[:, :, 5:8, b].rearrange("p h v -> p v h")
        for cp in range(NPAIR):
            nc.tensor.matmul(pb[:, cp * 16:cp * 16 + 10],
                             t16b[:, cp], rhs_m, start=True, stop=True)
        for cp in range(NPAIR):
            nc.tensor.matmul(pb[:, cp * 16 + 10:cp * 16 + 16],
                             t2b[:, cp], rhs_s, start=True, stop=True)
        # de-interleave (cp, 2v+h) -> (v, c=2cp+h): each stream c-contiguous
        pbb = pb[:, :]
        pb_src = bass.AP(tensor=pbb.tensor, offset=pbb.offset,
                         ap=[pbb.ap[0], [2, 8], [16, 32], [1, 2]])
        nc.vector.tensor_copy(
            qall[:, bb].rearrange("p (v c h) -> p v c h", v=8, h=2), pb_src)

    # ---------------- chain phases (split for interleaving) ----------------
    w_ws = {}
    wrv4s = {}

    def emit_chain_write(gi):
        gs = gi * GRP
        qall = qalls[gi]
        w_w = work.tile([P, GRP, CH], BF16, tag=f"w_w_g{gi}",
                        name=f"w_w_g{gi}")
        w_ws[gi] = (w_w, qall)
        w_chain_all(qall[:, :, 0:64], qall[:, :, 320:384],
                    pw_w[:, gs:gs + GRP], 0, gs, gi, w_w)

    def emit_chain_read(gi):
        gs = gi * GRP
        w_w, qall = w_ws[gi]

        def QV(q):
            return qall[:, :, 64 * q:64 * q + 64]

        # read-head inputs via algebra (QV: 0 k_w, 1 k_r, 2 e*k_r, 3 a,
        # 4 a*e, 5 ssm, 6 sme, 7 sme2)
        dots_r = ctile("dots_r", gi)
        t_a = ctile("alg_t", gi)
        nc.vector.scalar_tensor_tensor(t_a, QV(2), -1.0, scb3(Q_AKR, gs),
                                       op0=OP.mult, op1=OP.add)
        nc.vector.tensor_mul(t_a, w_w, t_a)
        nc.vector.tensor_add(dots_r, t_a, QV(1))

        ss_r = ctile("ss_r", gi)
        a1 = ctile("alg_a1", gi)
        nc.vector.tensor_sub(a1, QV(3), QV(6))  # sma - sme
        a2 = ctile("alg_a2", gi)
        nc.vector.scalar_tensor_tensor(a2, QV(4), -2.0, scb3(Q_AA, gs),
                                       op0=OP.mult, op1=OP.add)
        nc.vector.tensor_add(a2, a2, QV(7))  # + sme2
        h1 = ctile("alg_h1", gi)
        nc.vector.tensor_mul(h1, w_w, a2)
        nc.vector.scalar_tensor_tensor(h1, a1, 2.0, h1,
                                       op0=OP.mult, op1=OP.add)
        nc.vector.tensor_mul(h1, w_w, h1)
        nc.vector.tensor_add(ss_r, h1, QV(5))  # + ssm

        w_r = work.tile([P, GRP, CH], BF16, tag=f"w_r_g{gi}",
                        name=f"w_r_g{gi}")
        w_chain_all(dots_r, ss_r, pw_r[:, gs:gs + GRP],
                    8, gs, gi, w_r)

        # wrv4[p, bb, c, 0] = w_r ; [.., 1] = w_r*w_w  (bf16 for rv moving)
        wrv4 = work.tile([P, GRP, CH, 2], BF16, tag=f"wrv_g{gi}",
                         name=f"wrv_g{gi}")
        wrv4s[gi] = wrv4
        nc.vector.tensor_copy(wrv4[:, :, :, 0], w_r)
        wrw = ctile("wrw", gi)
        nc.vector.tensor_mul(wrw, w_r, w_w)
        nc.vector.tensor_copy(wrv4[:, :, :, 1], wrw)
        # swr[b] = sum_n w_r*w_w
        swc = gtile("swc", gi, F32)
        nc.vector.reduce_sum(swc, wrw, axis=mybir.AxisListType.X)
        ps_sw = ps_misc.tile([128, 144], F32, tag="pm")
        nc.tensor.matmul(ps_sw[0:GRP, 0:1], swc, ones_sb[:, 0:1],
                         start=True, stop=True)
        swr_c = work.tile([GRP, 1], F32, tag=f"swr_c_g{gi}",
                          name=f"swr_c_g{gi}")
        nc.vector.tensor_copy(swr_c, ps_sw[0:GRP, 0:1])
        ps_swt = ps_misc.tile([128, 144], F32, tag="pm")
        nc.tensor.transpose(ps_swt[0:1, 0:GRP], swr_c,
                            ident_sb[0:GRP, 0:GRP])
        gs2 = gi * GRP
        nc.vector.tensor_copy(swr_sb[:, gs2:gs2 + GRP], ps_swt[0:1, 0:GRP])

    def emit_rv(b):
        # rv contraction: memory chunk-pair [128, 128] stationary (FWL),
        # [w_r, w_r*w_w] for both chunks as 4-wide moving; chunk halves
        # land on partition halves of a [128, 4] accumulating PSUM.
        gi, bb = b // GRP, b % GRP
        wrv4 = wrv4s[gi]
        ps_rv = ps_rvp.tile([128, 4], F32, tag="ps_rv")
        for q in range(NPAIR):
            lhs = m16s[b][:, 2 * q:2 * q + 2, :].rearrange("p c d -> p (c d)")
            rhs = wrv4[:, bb, 2 * q:2 * q + 2, :].rearrange("p c j -> p (c j)")
            nc.tensor.matmul(ps_rv, lhs, rhs,
                             start=(q == 0), stop=(q == NPAIR - 1))
        # valid: partitions 0-63 <- cols 0:2 (chunk-even), 64-127 <- 2:4
        nc.vector.tensor_copy(r12_all[0:64, b, :], ps_rv[0:64, 0:2])
        nc.vector.tensor_copy(r12_all[64:128, b, :], ps_rv[64:128, 2:4])

    # ---------------- emission schedule (software pipeline) ----------------
    emit_batch(0)
    emit_batch(1)
    emit_batch(2)
    emit_batch(3)
    emit_chain_write(0)
    emit_batch(4)
    emit_chain_read(0)
    emit_batch(5)
    emit_rv(0)
    emit_rv(1)
    emit_batch(6)
    emit_rv(2)
    emit_rv(3)
    emit_batch(7)
    emit_chain_write(1)
    emit_chain_read(1)
    for b in range(4, 8):
        emit_rv(b)

    # ---------------- read-vector assembly (all batches) ----------------
    # rvt_full[(c',d), b] = r1 - e*r2 + a*swr  on both partition halves,
    # then one transpose and a free-axis fold of the halves.
    ps_swb = ps_misc.tile([128, 144], F32, tag="pm")
    nc.tensor.matmul(ps_swb[:, 0:BL], ones_sb[0:1, :], swr_sb,
                     start=True, stop=True)
    rvt = work.tile([128, BL], F32, tag="rvt", name="rvt")
    nc.vector.tensor_mul(rvt, e2_sb, r12_all[:, :, 1])   # e * r2
    nc.vector.tensor_sub(rvt, r12_all[:, :, 0], rvt)     # r1 - e*r2
    m3 = work.tile([128, BL], F32, tag="m3", name="m3")
    nc.vector.tensor_mul(m3, a2_sb, ps_swb[:, 0:BL])     # a * sum(wr*ww)
    nc.vector.tensor_add(rvt, rvt, m3)
    ps_rvo = ps_misc.tile([128, 144], F32, tag="pm")
    nc.tensor.transpose(ps_rvo[0:BL, 0:128], rvt, ident_sb)
    rvo_sb = work.tile([BL, 128], F32, tag="rvo_sb", name="rvo_sb")
    nc.vector.tensor_copy(rvo_sb, ps_rvo[0:BL, 0:128])
    nc.vector.tensor_add(out_sb[:, C:C + D], rvo_sb[:, 0:64],
                         rvo_sb[:, 64:128])

    nc.sync.dma_start(out=out_d[:], in_=out_sb)
    ctx.close()


# ---------------------------------------------------------------------------
# host-side driver
# ---------------------------------------------------------------------------
_NC = None


def _get_module():
    global _NC
    if _NC is None:
        _NC = _build_module()
    return _NC


def _consts():
    ident = np.eye(128, dtype=np.float32)
    onest = np.ones((128, 128), np.float32)
    permu = np.zeros((128, 128), np.float32)
    permd = np.zeros((128, 128), np.float32)
    for m in range(128):
        permu[(m + 1) % 128, m] = 1.0
        permd[(m - 1) % 128, m] = 1.0
    sel = np.zeros((32, NQ * 128), np.float32)
    for q in range(NQ):
        sel[q, q * 128:(q + 1) * 128] = 1.0
    return ident, onest, permu, permd, sel


def kernel(**inputs):
    from concourse.bass_utils import run_bass_kernel_spmd

    nc = _get_module()
    f = lambda k: np.ascontiguousarray(np.asarray(inputs[k], np.float32))

    whead = np.concatenate([
        f("Wk_r"), f("Wb_r"), f("Wg_r"), f("Ws_r"), f("Wgam_r"),
        f("Wk_w"), f("Wb_w"), f("Wg_w"), f("Ws_w"), f("Wgam_w"),
        f("We_w"), f("Wa_w")], axis=1)
    bhead = np.concatenate([
        f("bk_r"), f("bb_r"), f("bg_r"), f("bs_r"), f("bgam_r"),
        f("bk_w"), f("bb_w"), f("bg_w"), f("bs_w"), f("bgam_w"),
        f("be_w"), f("ba_w")])
    ident, onest, permu, permd, sel = _consts()

    mem = f("prev_memory")
    x = f("x")
    rv = f("prev_read_vector")
    prw = f("prev_read_weights")
    pww = f("prev_write_weights")
    shared = dict(wctrl=f("W_ctrl"), bctrl=f("b_ctrl"), whead=whead,
                  bhead=bhead, ident=ident, onest=onest, permu=permu,
                  permd=permd, sel=sel)
    in_maps = []
    for c in range(NCORES):
        sl = slice(c * BL, (c + 1) * BL)
        in_maps.append(dict(
            mem=np.ascontiguousarray(mem[sl]),
            x=np.ascontiguousarray(x[sl]),
            rv=np.ascontiguousarray(rv[sl]),
            prw=np.ascontiguousarray(prw[sl]),
            pww=np.ascontiguousarray(pww[sl]),
            **shared))
    res = run_bass_kernel_spmd(nc, in_maps, list(range(NCORES)))
    return np.concatenate([res.results[c]["out"] for c in range(NCORES)],
                          axis=0).astype(np.float32)


# revision 36
# speedup vs baseline: 1.0570x; 1.0332x over previous
"""NTM cell kernel for Trainium2 (8 NeuronCores, batch-parallel).

Strategy (per core, 8 batches):
  - prev_memory slice is cast-loaded f32->bf16 into SBUF (row-major M16).
  - The on-chip transpose to T16 runs on the TENSOR engine (128x128 bf16
    transposes into PSUM, ~1 cyc/row) instead of the DMA xbar, freeing the
    DMA pool for the HBM load; PSUM->SBUF cast copies alternate between
    the vector and scalar engines, elementwise squares (T2) between
    gpsimd and vector.
  - All O(N*D) reductions run on the tensor engine:
      * content dots + sum-of-squares streams over T16 / T2 (stationary
        rides the FWL weight path at ~0.5 cyc/col)
      * read-vector contraction with the memory chunk-pair as the
        128-col stationary and [w_r, w_r*w_w] as a 4-wide moving; the
        two chunk-halves land on partition halves and are folded after
        the final output transpose.
  - new_memory is never materialized; its dot/norm/read contributions are
    expanded algebraically in terms of streams over the ORIGINAL memory.
  - Addressing chains (softmax/gate/shift/sharpen) run on DVE/ACT/GPSIMD
    in a [128 x 64] layout (n = p*64 + c), with per-group buffers so the
    two batch-groups' chains pipeline instead of serializing.
  - Only one ACT table set is used (exp/ln); sqrt/sigmoid/tanh/softplus
    are rewritten via exp/ln so no table reloads occur.
"""

import sys

sys.path.insert(0, "/opt/trn_rl_repo")

import numpy as np

import concourse.bass as bass
import concourse.tile as tile
from concourse import mybir

F32 = mybir.dt.float32
BF16 = mybir.dt.bfloat16
AF = mybir.ActivationFunctionType
OP = mybir.AluOpType

B, N, D, C, IN, S = 64, 8192, 64, 256, 128, 3
NCORES = 8
BL = B // NCORES          # batches per core
P = 128                   # partitions
CH = N // P               # 64 chunks per batch (n = p*64 + c)
NPAIR = CH // 2           # 32 transposed tiles per batch
EPS = 1e-8

# whead column map
KR0, KR1 = 0, 64
BR, GR = 64, 65
SR0, SR1 = 66, 69
GAMR = 69
KW0, KW1 = 70, 134
BW, GW = 134, 135
SW0, SW1 = 136, 139
GAMW = 139
E0, E1 = 140, 204
A0, A1 = 204, 268
NHEAD = 268

# scalar table rows (S8 cols -> SC rows -> BC blocks of 8)
Q_BET_W, Q_G_W, Q_OMG_W, Q_SW0, Q_SW1, Q_SW2, Q_GAM_W, Q_NK2_W = range(8)
Q_BET_R, Q_G_R, Q_OMG_R, Q_SR0, Q_SR1, Q_SR2, Q_GAM_R, Q_NK2_R = range(8, 16)
Q_AKR, Q_AA = 16, 17
NQ = 18

GRP = 4  # batches per pipeline group

# ---------------------------------------------------------------------------
# workaround: the deployed walrus accepts only ONE sem-wait per instruction.
# After TileContext exits, hoist extra waits onto injected single-wait nops
# (drains on the SP engine, ENGINE_NOPs elsewhere).
# ---------------------------------------------------------------------------
import concourse.tile as tile_mod


def _split_multi_waits(nc):
    for f in nc.m.functions:
        for b in f.blocks:
            insts = b.instructions
            i = 0
            while i < len(insts):
                ins = insts[i]
                si = getattr(ins, "sync_info", None)
                if si is None or len(si.on_wait) <= 1:
                    i += 1
                    continue
                waits = list(si.on_wait)
                ins.sync_info = mybir.SyncInfo(
                    on_wait=[waits[-1]], on_update=list(si.on_update)
                )
                eng = nc.engines[ins.engine]
                new_nops = []
                for w in waits[:-1]:
                    nop = eng.isa(
                        nc.isa.Opcode.NEURON_ISA_TPB_OPCODE_NOP, {}
                    ).ins
                    nop.sync_info = mybir.SyncInfo(on_wait=[w], on_update=[])
                    new_nops.append(nop)
                for nop in new_nops:
                    for bb2 in f.blocks:
                        try:
                            bb2.instructions.remove(nop)
                            break
                        except ValueError:
                            pass
                for k, nop in enumerate(new_nops):
                    insts.insert(i + k, nop)
                i += len(new_nops) + 1


_orig_exit = tile_mod.TileContext.__exit__


def _patched_exit(self, *a, **k):
    import os
    r = _orig_exit(self, *a, **k)
    if not os.environ.get("NTM_NO_WAITFIX"):
        _split_multi_waits(self.nc)
    return r


if not getattr(tile_mod.TileContext, "_waitfix_patched", False):
    tile_mod.TileContext.__exit__ = _patched_exit
    tile_mod.TileContext._waitfix_patched = True


# ---------------------------------------------------------------------------
# kernel body
# ---------------------------------------------------------------------------

def _build_module():
    nc = bass.Bass()

    mem = nc.dram_tensor("mem", [BL, N, D], F32, kind="ExternalInput")
    x_in = nc.dram_tensor("x", [BL, IN], F32, kind="ExternalInput")
    rv_in = nc.dram_tensor("rv", [BL, D], F32, kind="ExternalInput")
    prw_in = nc.dram_tensor("prw", [BL, N], F32, kind="ExternalInput")
    pww_in = nc.dram_tensor("pww", [BL, N], F32, kind="ExternalInput")
    wctrl = nc.dram_tensor("wctrl", [IN + D, C], F32, kind="ExternalInput")
    bctrl = nc.dram_tensor("bctrl", [C], F32, kind="ExternalInput")
    whead = nc.dram_tensor("whead", [C, NHEAD], F32, kind="ExternalInput")
    bhead = nc.dram_tensor("bhead", [NHEAD], F32, kind="ExternalInput")
    ident = nc.dram_tensor("ident", [128, 128], F32, kind="ExternalInput")
    onest = nc.dram_tensor("onest", [128, 128], F32, kind="ExternalInput")
    permu = nc.dram_tensor("permu", [128, 128], F32, kind="ExternalInput")
    permd = nc.dram_tensor("permd", [128, 128], F32, kind="ExternalInput")
    seldr = nc.dram_tensor("sel", [32, NQ * 128], F32, kind="ExternalInput")
    out_d = nc.dram_tensor("out", [BL, C + D], F32, kind="ExternalOutput")

    with tile.TileContext(nc) as tc:
        _emit(nc, tc, mem, x_in, rv_in, prw_in, pww_in, wctrl, bctrl, whead,
              bhead, ident, onest, permu, permd, seldr, out_d)
    return nc


def _emit(nc, tc, mem, x_in, rv_in, prw_in, pww_in, wctrl, bctrl, whead,
          bhead, ident, onest, permu, permd, seldr, out_d):
    from contextlib import ExitStack

    ctx = ExitStack()
    ctx.enter_context(nc.allow_low_precision(
        reason="bf16 chain intermediates; rel-err budget 2e-2"))
    big = ctx.enter_context(tc.tile_pool(name="big", bufs=1))
    cons = ctx.enter_context(tc.tile_pool(name="cons", bufs=1))
    work = ctx.enter_context(tc.tile_pool(name="work", bufs=1))
    t16p = ctx.enter_context(tc.tile_pool(name="t16p", bufs=3))
    t2p = ctx.enter_context(tc.tile_pool(name="t2p", bufs=2))
    qallp = ctx.enter_context(tc.tile_pool(name="qallp", bufs=2))
    ps_tr = ctx.enter_context(tc.tile_pool(name="ps_tr", bufs=2, space="PSUM"))
    ps_stream = ctx.enter_context(tc.tile_pool(name="ps_stream", bufs=2, space="PSUM"))
    ps_misc = ctx.enter_context(tc.tile_pool(name="ps_misc", bufs=3, space="PSUM"))
    ps_rvp = ctx.enter_context(tc.tile_pool(name="ps_rvp", bufs=1, space="PSUM"))

    # ---------------- big memory load: issue FIRST ----------------
    # batch 0 on sync so its descriptors hit the rings first; the rest
    # sequentially behind it on gpsimd (per-ring FIFO keeps completion
    # roughly batch-ordered).
    # bf16 const cast-loads go FIRST on the gpsimd SW queue (tiny), then
    # the 8 per-batch memory cast-loads (f32->bf16 in the DGE) behind them.
    m16s = [big.tile([P, CH, D], BF16, tag=f"m16_{b}", name=f"m16_{b}")
            for b in range(BL)]

    identb_sb = cons.tile([128, 128], BF16, tag="identb")
    nc.gpsimd.dma_start(out=identb_sb, in_=ident[:])
    wh0 = cons.tile([128, NHEAD], BF16, tag="wh0")
    nc.gpsimd.dma_start(out=wh0, in_=whead[0:128, :])
    wh1 = cons.tile([128, NHEAD], BF16, tag="wh1")
    nc.gpsimd.dma_start(out=wh1, in_=whead[128:256, :])
    bh_sb = cons.tile([1, NHEAD], BF16, tag="bh")
    nc.gpsimd.dma_start(out=bh_sb, in_=bhead.rearrange("(o n) -> o n", o=1))
    selb_sb = cons.tile([32, NQ * 128], BF16, tag="selb")
    nc.gpsimd.dma_start(out=selb_sb, in_=seldr[:])
    onesb_sb = cons.tile([1, 128], BF16, tag="onesb")
    nc.gpsimd.dma_start(out=onesb_sb, in_=onest[0:1, :])
    permub_sb = cons.tile([128, 128], BF16, tag="permub")
    nc.gpsimd.dma_start(out=permub_sb, in_=permu[:])
    permdb_sb = cons.tile([128, 128], BF16, tag="permdb")
    nc.gpsimd.dma_start(out=permdb_sb, in_=permd[:])
    for b in range(BL):
        nc.gpsimd.dma_start(
            out=m16s[b], in_=mem[b].rearrange("(p c) d -> p c d", p=128)
        )

    wc0 = cons.tile([128, C], F32, tag="wc0")
    nc.sync.dma_start(out=wc0, in_=wctrl[0:128, :])
    wc1 = cons.tile([64, C], F32, tag="wc1")
    nc.sync.dma_start(out=wc1, in_=wctrl[128:192, :])
    bc_sb = cons.tile([128, 2], F32, tag="bc")
    nc.sync.dma_start(out=bc_sb, in_=bctrl.rearrange("(j p) -> p j", p=128))
    xt_in = cons.tile([BL, IN], F32, tag="xt_in")
    nc.sync.dma_start(out=xt_in, in_=x_in[:])
    rv_sb = cons.tile([BL, D], F32, tag="rv_sb")
    nc.sync.dma_start(out=rv_sb, in_=rv_in[:])
    ident_sb = cons.tile([128, 128], F32, tag="ident")
    nc.sync.dma_start(out=ident_sb, in_=ident[:])

    ones_sb = cons.tile([128, 128], F32, tag="ones")
    nc.scalar.dma_start(out=ones_sb, in_=onest[:])
    permu_sb = cons.tile([128, 128], F32, tag="permu")
    nc.scalar.dma_start(out=permu_sb, in_=permu[:])
    permd_sb = cons.tile([128, 128], F32, tag="permd")
    nc.scalar.dma_start(out=permd_sb, in_=permd[:])
    pw_w = cons.tile([128, BL, CH], F32, tag="pw_w")
    nc.scalar.dma_start(out=pw_w, in_=pww_in.rearrange("b (p c) -> p b c", p=128))
    pw_r = cons.tile([128, BL, CH], F32, tag="pw_r")
    nc.scalar.dma_start(out=pw_r, in_=prw_in.rearrange("b (p c) -> p b c", p=128))


    # ---------------- controller: hT = relu(W_ctrl^T @ ctrl_in^T + b) -------
    ps_xt = ps_misc.tile([128, 144], F32, tag="pm")
    nc.tensor.transpose(ps_xt[:, 0:BL], xt_in, ident_sb[0:BL, 0:BL])
    xT = work.tile([128, BL], F32, tag="xT")
    nc.vector.tensor_copy(xT, ps_xt[:, 0:BL])
    ps_rt = ps_misc.tile([128, 144], F32, tag="pm")
    nc.tensor.transpose(ps_rt[0:D, 0:BL], rv_sb, ident_sb[0:BL, 0:BL])
    rvT = work.tile([64, BL], F32, tag="rvT")
    nc.vector.tensor_copy(rvT, ps_rt[0:D, 0:BL])

    hT_sb = []
    for j in range(2):
        ps_h = ps_misc.tile([128, 144], F32, tag="pm")
        nc.tensor.matmul(ps_h[:, 0:BL], wc0[:, j * 128:(j + 1) * 128], xT,
                         start=True, stop=False)
        nc.tensor.matmul(ps_h[:, 0:BL], wc1[:, j * 128:(j + 1) * 128], rvT,
                         start=False, stop=True)
        h_j = work.tile([128, BL], F32, tag=f"hT{j}")
        nc.scalar.activation(h_j, ps_h[:, 0:BL], AF.Relu,
                             bias=bc_sb[:, j:j + 1], scale=1.0)
        hT_sb.append(h_j)

    # ---------------- head params P = h @ Whead + bhead (bf16) ----------
    hT_b = []
    for j in range(2):
        hb = work.tile([128, BL], BF16, tag=f"hTb{j}", name=f"hTb{j}")
        nc.vector.tensor_copy(hb, hT_sb[j])
        hT_b.append(hb)
    ps_p = ps_misc.tile([BL, 512], F32, tag="pm")
    nc.tensor.matmul(ps_p[:, 0:NHEAD], hT_b[0], wh0, start=True, stop=False)
    nc.tensor.matmul(ps_p[:, 0:NHEAD], hT_b[1], wh1, start=False, stop=False)
    nc.tensor.matmul(ps_p[:, 0:NHEAD], onesb_sb[0:1, 0:BL], bh_sb,
                     start=False, stop=True)
    p_sb = work.tile([BL, NHEAD], F32, tag="p_sb")
    nc.vector.tensor_copy(p_sb, ps_p[:, 0:NHEAD])

    # ---------------- VA: per-batch d-vectors [BL, 8*64] ----------------
    # vec order: 0 k_w, 1 k_r, 2 e*k_r, 3 a, 4 a*e, 5 ones, 6 e, 7 e^2
    va = work.tile([BL, 512], F32, tag="va")
    nc.vector.tensor_copy(va[:, 0:64], p_sb[:, KW0:KW1])
    nc.vector.tensor_copy(va[:, 64:128], p_sb[:, KR0:KR1])

    def _sigmoid(dst, src):  # dst = 1/(1+exp(-src))
        nc.scalar.activation(dst, src, AF.Exp, scale=-1.0)
        nc.vector.tensor_scalar_add(dst, dst, 1.0)
        nc.vector.reciprocal(dst, dst)

    # e = sigmoid(P_e) -> va[:, 384:448]
    _sigmoid(va[:, 384:448], p_sb[:, E0:E1])
    # a = tanh(P_a) = 1 - 2/(exp(2x)+1) -> va[:, 192:256]
    nc.scalar.activation(va[:, 192:256], p_sb[:, A0:A1], AF.Exp, scale=2.0)
    nc.vector.tensor_scalar_add(va[:, 192:256], va[:, 192:256], 1.0)
    nc.vector.reciprocal(va[:, 192:256], va[:, 192:256])
    nc.vector.tensor_scalar(va[:, 192:256], va[:, 192:256], -2.0, 1.0,
                            op0=OP.mult, op1=OP.add)
    # beta' = softplus(P_beta) * rsqrt(||k||^2): folded into the stream
    # vectors so the chain's beta-mul and nk2-normalization disappear.
    tmp64p = work.tile([BL, 64], F32, tag="tmp64p")
    bp = work.tile([BL, 4], F32, tag="bp")
    nc.vector.tensor_mul(tmp64p, va[:, 0:64], va[:, 0:64])
    nc.vector.reduce_sum(bp[:, 0:1], tmp64p, axis=mybir.AxisListType.X)
    nc.vector.tensor_mul(tmp64p, va[:, 64:128], va[:, 64:128])
    nc.vector.reduce_sum(bp[:, 1:2], tmp64p, axis=mybir.AxisListType.X)
    nc.scalar.activation(bp[:, 0:2], bp[:, 0:2], AF.Ln)
    nc.scalar.activation(bp[:, 0:2], bp[:, 0:2], AF.Exp, scale=-0.5)
    nc.scalar.activation(bp[:, 2:3], p_sb[:, BW:BW + 1], AF.Exp)
    nc.scalar.activation(bp[:, 3:4], p_sb[:, BR:BR + 1], AF.Exp)
    nc.vector.tensor_scalar_add(bp[:, 2:4], bp[:, 2:4], 1.0)
    nc.scalar.activation(bp[:, 2:4], bp[:, 2:4], AF.Ln)
    nc.vector.tensor_mul(bp[:, 2:3], bp[:, 2:3], bp[:, 0:1])
    nc.vector.tensor_mul(bp[:, 3:4], bp[:, 3:4], bp[:, 1:2])
    nc.vector.tensor_scalar(va[:, 0:64], va[:, 0:64], bp[:, 2:3], None,
                            op0=OP.mult)
    nc.vector.tensor_scalar(va[:, 64:128], va[:, 64:128], bp[:, 3:4], None,
                            op0=OP.mult)
    # e*k_r (scaled), a*e, ones, e^2
    nc.vector.tensor_mul(va[:, 128:192], va[:, 384:448], va[:, 64:128])
    nc.vector.tensor_mul(va[:, 256:320], va[:, 192:256], va[:, 384:448])
    nc.vector.memset(va[:, 320:384], 1.0)
    nc.vector.tensor_mul(va[:, 448:512], va[:, 384:448], va[:, 384:448])

    # ---------------- VTD: transposed vectors with zero-halves --------------
    # VTD[p, half, vec, b]; half 0: rows 0-63 hold vec, rows 64-127 zero.
    vtd = work.tile([128, 2, 8, BL], BF16, tag="vtd")
    nc.vector.memset(vtd, 0.0)
    vapad = work.tile([BL, 8, 128], F32, tag="vapad")
    nc.vector.memset(vapad, 0.0)
    for v in range(8):
        nc.vector.tensor_copy(vapad[:, v, 64:128], va[:, v * 64:(v + 1) * 64])
    ps_top = ps_misc.tile([128, 144], F32, tag="pm")
    ps_bot = ps_misc.tile([128, 144], F32, tag="pm")
    for v in range(8):
        nc.tensor.transpose(ps_top[0:64, v * BL:(v + 1) * BL],
                            va[:, v * 64:(v + 1) * 64],
                            ident_sb[0:BL, 0:BL])
        nc.tensor.transpose(ps_bot[:, v * BL:(v + 1) * BL],
                            vapad[:, v, :], ident_sb[0:BL, 0:BL])
    nc.vector.tensor_copy(
        vtd[0:64].rearrange("p h v b -> p (h v b)")[:, 0:64],
        ps_top[0:64, 0:64])
    nc.vector.tensor_copy(
        vtd[64:128].rearrange("p h v b -> p (h v b)")[:, 64:128],
        ps_bot[64:128, 0:64])
    # e/a duplicated across both partition halves for the rv assembly:
    # ea_dup[b, 0:64] = e, [64:128] = e  (same for a) -> transpose -> [128, BL]
    ea_dup = work.tile([BL, 2, 128], F32, tag="ea_dup")
    nc.vector.tensor_copy(ea_dup[:, 0, 0:64], va[:, 384:448])
    nc.vector.tensor_copy(ea_dup[:, 0, 64:128], va[:, 384:448])
    nc.vector.tensor_copy(ea_dup[:, 1, 0:64], va[:, 192:256])
    nc.vector.tensor_copy(ea_dup[:, 1, 64:128], va[:, 192:256])
    ps_ea = ps_misc.tile([128, 144], F32, tag="pm")
    nc.tensor.transpose(ps_ea[:, 0:BL], ea_dup[:, 0, :], ident_sb[0:BL, 0:BL])
    nc.tensor.transpose(ps_ea[:, BL:2 * BL], ea_dup[:, 1, :],
                        ident_sb[0:BL, 0:BL])
    e2_sb = work.tile([128, BL], F32, tag="e2_sb")
    nc.vector.tensor_copy(e2_sb, ps_ea[:, 0:BL])
    a2_sb = work.tile([128, BL], F32, tag="a2_sb")
    nc.vector.tensor_copy(a2_sb, ps_ea[:, BL:2 * BL])

    # ---------------- per-batch scalars S8 [BL, 32] ----------------
    s8 = work.tile([BL, 32], F32, tag="s8")
    nc.vector.memset(s8, 0.0)
    tmp64 = work.tile([BL, 64], F32, tag="tmp64")

    def _softplus(dst, src):  # ln(1 + exp(src))
        nc.scalar.activation(dst, src, AF.Exp)
        nc.vector.tensor_scalar_add(dst, dst, 1.0)
        nc.scalar.activation(dst, dst, AF.Ln)

    def _softmax3(dst, src):
        ex3 = work.tile([BL, 3], F32, tag="ex3")
        nc.scalar.activation(ex3, src, AF.Exp)
        sm = work.tile([BL, 1], F32, tag="sm3")
        nc.vector.reduce_sum(sm, ex3, axis=mybir.AxisListType.X)
        nc.vector.reciprocal(sm, sm)
        nc.vector.tensor_scalar(dst, ex3, sm, None, op0=OP.mult)

    _softplus(s8[:, Q_BET_W:Q_BET_W + 1], p_sb[:, BW:BW + 1])
    _sigmoid(s8[:, Q_G_W:Q_G_W + 1], p_sb[:, GW:GW + 1])
    nc.vector.tensor_scalar(s8[:, Q_OMG_W:Q_OMG_W + 1],
                            s8[:, Q_G_W:Q_G_W + 1], -1.0, 1.0,
                            op0=OP.mult, op1=OP.add)
    _softmax3(s8[:, Q_SW0:Q_SW0 + 3], p_sb[:, SW0:SW1])
    _softplus(s8[:, Q_GAM_W:Q_GAM_W + 1], p_sb[:, GAMW:GAMW + 1])
    nc.vector.tensor_scalar_add(s8[:, Q_GAM_W:Q_GAM_W + 1],
                                s8[:, Q_GAM_W:Q_GAM_W + 1], 1.0)
    nc.vector.tensor_mul(tmp64, va[:, 0:64], va[:, 0:64])
    nc.vector.reduce_sum(s8[:, Q_NK2_W:Q_NK2_W + 1], tmp64,
                         axis=mybir.AxisListType.X)

    _softplus(s8[:, Q_BET_R:Q_BET_R + 1], p_sb[:, BR:BR + 1])
    _sigmoid(s8[:, Q_G_R:Q_G_R + 1], p_sb[:, GR:GR + 1])
    nc.vector.tensor_scalar(s8[:, Q_OMG_R:Q_OMG_R + 1],
                            s8[:, Q_G_R:Q_G_R + 1], -1.0, 1.0,
                            op0=OP.mult, op1=OP.add)
    _softmax3(s8[:, Q_SR0:Q_SR0 + 3], p_sb[:, SR0:SR1])
    _softplus(s8[:, Q_GAM_R:Q_GAM_R + 1], p_sb[:, GAMR:GAMR + 1])
    nc.vector.tensor_scalar_add(s8[:, Q_GAM_R:Q_GAM_R + 1],
                                s8[:, Q_GAM_R:Q_GAM_R + 1], 1.0)
    nc.vector.tensor_mul(tmp64, va[:, 64:128], va[:, 64:128])
    nc.vector.reduce_sum(s8[:, Q_NK2_R:Q_NK2_R + 1], tmp64,
                         axis=mybir.AxisListType.X)

    nc.vector.tensor_mul(tmp64, va[:, 192:256], va[:, 64:128])
    nc.vector.reduce_sum(s8[:, Q_AKR:Q_AKR + 1], tmp64,
                         axis=mybir.AxisListType.X)
    nc.vector.tensor_mul(tmp64, va[:, 192:256], va[:, 192:256])
    nc.vector.reduce_sum(s8[:, Q_AA:Q_AA + 1], tmp64,
                         axis=mybir.AxisListType.X)

    # transpose S8 -> SC [32, BL] and broadcast -> BC [128, NQ*8]
    ps_sc = ps_misc.tile([128, 144], F32, tag="pm")
    nc.tensor.transpose(ps_sc[0:32, 0:BL], s8, ident_sb[0:BL, 0:BL])
    sc_sb = work.tile([32, BL], BF16, tag="sc_sb")
    nc.vector.tensor_copy(sc_sb, ps_sc[0:32, 0:BL])
    ps_bc = ps_misc.tile([128, 144], F32, tag="pm")
    for q in range(NQ):
        nc.tensor.matmul(ps_bc[:, q * BL:(q + 1) * BL],
                         selb_sb[:, q * 128:(q + 1) * 128], sc_sb,
                         start=True, stop=True)
    bc_all = work.tile([128, NQ * BL], BF16, tag="bc_all")
    nc.vector.tensor_copy(bc_all, ps_bc[:, 0:NQ * BL])
    bc_f32 = work.tile([128, NQ * BL], F32, tag="bc_f32")
    nc.vector.tensor_copy(bc_f32, ps_bc[:, 0:NQ * BL])

    def BCF(q, b):
        return bc_f32[:, q * BL + b:q * BL + b + 1]

    def BC(q, b):
        return bc_all[:, q * BL + b:q * BL + b + 1]

    # ---------------- output staging ----------------
    out_sb = work.tile([BL, C + D], F32, tag="out_sb")
    ps_ho = ps_misc.tile([128, 144], F32, tag="pm")
    nc.tensor.transpose(ps_ho[0:BL, 0:128], hT_sb[0], ident_sb)
    nc.vector.tensor_copy(out_sb[:, 0:128], ps_ho[0:BL, 0:128])
    ps_ho2 = ps_misc.tile([128, 144], F32, tag="pm")
    nc.tensor.transpose(ps_ho2[0:BL, 0:128], hT_sb[1], ident_sb)
    nc.vector.tensor_copy(out_sb[:, 128:256], ps_ho2[0:BL, 0:128])

    swr_sb = work.tile([1, BL], F32, tag="swr_sb")
    r12_all = work.tile([128, BL, 2], F32, tag="r12_all")

    # ---------------- helpers for grouped heavy phase ----------------
    def scb3(q, gs):
        base = bc_all[:, q * BL + gs:q * BL + gs + GRP]
        return bass.AP(tensor=base.tensor, offset=base.offset,
                       ap=[base.ap[0], base.ap[1], [0, CH]])

    def scb3n(q, gs, n):
        base = bc_all[:, q * BL + gs:q * BL + gs + GRP]
        return bass.AP(tensor=base.tensor, offset=base.offset,
                       ap=[base.ap[0], base.ap[1], [0, n]])

    def bc3(t8):
        base = t8[:, :]
        return bass.AP(tensor=base.tensor, offset=base.offset,
                       ap=[base.ap[0], base.ap[1], [0, CH]])

    def ctile(tag, gi):
        tg = f"{tag}_g{gi}"
        return work.tile([P, GRP, CH], BF16, tag=tg, name=tg)

    def gtile(tag, gi, dt=F32):
        tg = f"{tag}_g{gi}"
        return work.tile([128, GRP], dt, tag=tg, name=tg)

    def psum_colsum_bcast(cs8, gi, eps=None, tag="tot", dt=F32):
        # one matmul with a full ones stationary both sums over partitions
        # and broadcasts the per-batch total to every output partition
        ps_t = ps_misc.tile([128, 144], F32, tag="pm")
        nc.tensor.matmul(ps_t[:, 0:GRP], ones_sb, cs8, start=True, stop=True)
        rt = gtile(tag + "_rt", gi, dt)
        if eps is not None:
            nc.vector.tensor_scalar_add(rt, ps_t[:, 0:GRP], eps)
            nc.vector.reciprocal(rt, rt)
        else:
            nc.vector.reciprocal(rt, ps_t[:, 0:GRP])
        return rt

    def w_chain_all(dk_v, ssm_v, pw_all, qo, gs, gi, dst):
        bet, g_, omg, s0, s1, s2, gam, nk2 = (qo + i for i in range(8))
        v = ctile("wc_v", gi)
        nc.scalar.activation(v, ssm_v, AF.Ln)
        inv = ctile("wc_inv", gi)
        nc.scalar.activation(inv, v, AF.Exp, scale=-0.5)
        bsim = ctile("wc_bsim", gi)
        nc.vector.tensor_mul(bsim, dk_v, inv)
        # exp + per-partition row-sum fused on ACT, one per batch
        ex = ctile("wc_ex", gi)
        cs = gtile("wc_cs", gi, F32)
        nc.scalar.activation(ex, bsim, AF.Exp)
        nc.vector.reduce_sum(cs, ex, axis=mybir.AxisListType.X)
        # sharpening is scale-invariant, so fold the content-softmax
        # denominator T into the interpolation instead of normalizing:
        # ws' = g*ex + T*(1-g)*pw  (T broadcast by the colsum matmul)
        ps_T = ps_misc.tile([128, 144], F32, tag="pm")
        nc.tensor.matmul(ps_T[:, 0:GRP], ones_sb, cs, start=True, stop=True)
        omgT = gtile("wc_omgT", gi)
        nc.vector.tensor_mul(omgT, ps_T[:, 0:GRP],
                             bc_all[:, omg * BL + gs:omg * BL + gs + GRP])
        t9 = ctile("wc_t9", gi)
        nc.vector.tensor_mul(t9, pw_all, bc3(omgT))
        wg = ctile("wc_wg", gi)
        for j in range(GRP):
            nc.scalar.activation(wg[:, j], ex[:, j], AF.Copy,
                                 scale=BCF(g_, gs + j))
        nc.vector.tensor_add(wg, wg, t9)
        # circular shift: body via shifted APs, boundary cols via perm matmuls
        ps_sh = ps_misc.tile([128, 144], F32, tag="pm")
        nc.tensor.matmul(ps_sh[:, 0:GRP], permub_sb, wg[:, :, 0],
                         start=True, stop=True)
        nc.tensor.matmul(ps_sh[:, GRP:2 * GRP], permdb_sb, wg[:, :, CH - 1],
                         start=True, stop=True)
        ws = ctile("wc_ws", gi)
        for j in range(GRP):
            nc.scalar.activation(ws[:, j], wg[:, j], AF.Copy,
                                 scale=BCF(s1, gs + j))
        tA = ctile("wc_tA", gi)
        nc.vector.tensor_mul(tA[:, :, 0:CH - 1], wg[:, :, 1:CH],
                             scb3n(s0, gs, CH - 1))
        nc.vector.tensor_add(ws[:, :, 0:CH - 1], ws[:, :, 0:CH - 1],
                             tA[:, :, 0:CH - 1])
        nc.vector.tensor_mul(tA[:, :, 1:CH], wg[:, :, 0:CH - 1],
                             scb3n(s2, gs, CH - 1))
        nc.vector.tensor_add(ws[:, :, 1:CH], ws[:, :, 1:CH],
                             tA[:, :, 1:CH])
        bnd = work.tile([128, 2 * GRP], F32, tag=f"wc_bnd_g{gi}",
                        name=f"wc_bnd_g{gi}")
        nc.vector.tensor_mul(bnd[:, 0:GRP], ps_sh[:, 0:GRP],
                             bc_all[:, s0 * BL + gs:s0 * BL + gs + GRP])
        nc.vector.tensor_mul(bnd[:, GRP:2 * GRP], ps_sh[:, GRP:2 * GRP],
                             bc_all[:, s2 * BL + gs:s2 * BL + gs + GRP])
        nc.vector.tensor_add(ws[:, :, CH - 1], ws[:, :, CH - 1],
                             bnd[:, 0:GRP])
        nc.vector.tensor_add(ws[:, :, 0], ws[:, :, 0], bnd[:, GRP:2 * GRP])
        # sharpening: wp = exp(gam * ln(ws)) with fused row-sums
        lg = ctile("wc_lg", gi)
        nc.scalar.activation(lg, ws, AF.Ln)
        wp = ctile("wc_wp", gi)
        cs2 = gtile("wc_cs2", gi, F32)
        for j in range(GRP):
            nc.scalar.activation(wp[:, j], lg[:, j], AF.Exp,
                                 scale=BCF(gam, gs + j))
        nc.vector.reduce_sum(cs2, wp, axis=mybir.AxisListType.X)
        rt2 = psum_colsum_bcast(cs2, gi, eps=EPS, tag="wc_t2", dt=BF16)
        nc.vector.tensor_mul(dst, wp, bc3(rt2))

    # ---------------- per-batch heavy stream ----------------
    qalls = {}

    def emit_batch(b):
        gi, bb = b // GRP, b % GRP
        if bb == 0:
            qalls[gi] = qallp.tile([P, GRP, 512], BF16, tag="qall",
                                   name="qall")
        qall = qalls[gi]
        t16b = t16p.tile([P, NPAIR, 128], BF16, tag="t16b", name="t16b")
        t2b = t2p.tile([P, NPAIR, 128], BF16, tag="t2b", name="t2b")
        m16f = m16s[b].rearrange("p c d -> p (c d)")
        if b == BL - 1:
            # the DMA pool is idle once the load drains: xbar-transpose the
            # last batch to take its transposes+copies off the PE/ACT path
            nc.sync.dma_start_transpose(t16b[:, 0:16], m16f[:, 0:2048])
            nc.scalar.dma_start_transpose(t16b[:, 16:32], m16f[:, 2048:4096])
            big0 = t16b[:, 0:16].rearrange("p a q -> p (a q)")
            big0d = t2b[:, 0:16].rearrange("p a q -> p (a q)")
            nc.gpsimd.tensor_mul(big0d, big0, big0)
            big1 = t16b[:, 16:32].rearrange("p a q -> p (a q)")
            big1d = t2b[:, 16:32].rearrange("p a q -> p (a q)")
            nc.vector.tensor_mul(big1d, big1, big1)
            _emit_streams(b, bb, qall, t16b, t2b)
            return
        # transposes in quads sharing one PSUM bank; copies alternate
        # vector/scalar, squares alternate gpsimd/vector
        for q in range(4):
            ps_t = ps_tr.tile([128, 1024], BF16, tag="ps_t")
            for k in range(8):
                cp = q * 8 + k
                nc.tensor.transpose(ps_t[:, k * 128:(k + 1) * 128],
                                    m16f[:, cp * 128:(cp + 1) * 128],
                                    identb_sb)
            t16v = t16b[:, q * 8:(q + 1) * 8].rearrange("p a q -> p (a q)")
            t2v = t2b[:, q * 8:(q + 1) * 8].rearrange("p a q -> p (a q)")
            if q == 2:
                nc.vector.tensor_copy(t16v, ps_t)
            else:
                nc.scalar.activation(t16v, ps_t, AF.Copy)
            if q == 1:
                big2 = t16b[:, 0:16].rearrange("p a q -> p (a q)")
                big2d = t2b[:, 0:16].rearrange("p a q -> p (a q)")
                nc.gpsimd.tensor_mul(big2d, big2, big2)
            elif q >= 2:
                nc.vector.tensor_mul(t2v, t16v, t16v)
        _emit_streams(b, bb, qall, t16b, t2b)

    def _emit_streams(b, bb, qall, t16b, t2b):
        pb = ps_stream.tile([128, 512], F32, tag="pb")
        rhs_m = vtd[:, :, 0:5, b].rearrange("p h v -> p v h")
        rhs_s = vtd[:, :, 5:8, b].rearrange("p h v -> p v h")
        for cp in range(NPAIR):
            nc.tensor.matmul(pb[:, cp * 16:cp * 16 + 10],
                             t16b[:, cp], rhs_m, start=True, stop=True)
        for cp in range(NPAIR):
            nc.tensor.matmul(pb[:, cp * 16 + 10:cp * 16 + 16],
                             t2b[:, cp], rhs_s, start=True, stop=True)
        # de-interleave (cp, 2v+h) -> (v, c=2cp+h): each stream c-contiguous
        pbb = pb[:, :]
        pb_src = bass.AP(tensor=pbb.tensor, offset=pbb.offset,
                         ap=[pbb.ap[0], [2, 8], [16, 32], [1, 2]])
        nc.vector.tensor_copy(
            qall[:, bb].rearrange("p (v c h) -> p v c h", v=8, h=2), pb_src)

    # ---------------- chain phases (split for interleaving) ----------------
    w_ws = {}
    wrv4s = {}

    def emit_chain_write(gi):
        gs = gi * GRP
        qall = qalls[gi]
        w_w = work.tile([P, GRP, CH], BF16, tag=f"w_w_g{gi}",
                        name=f"w_w_g{gi}")
        w_ws[gi] = (w_w, qall)
        w_chain_all(qall[:, :, 0:64], qall[:, :, 320:384],
                    pw_w[:, gs:gs + GRP], 0, gs, gi, w_w)

    def emit_chain_read(gi):
        gs = gi * GRP
        w_w, qall = w_ws[gi]

        def QV(q):
            return qall[:, :, 64 * q:64 * q + 64]

        # read-head inputs via algebra (QV: 0 k_w, 1 k_r, 2 e*k_r, 3 a,
        # 4 a*e, 5 ssm, 6 sme, 7 sme2)
        dots_r = ctile("dots_r", gi)
        t_a = ctile("alg_t", gi)
        nc.vector.scalar_tensor_tensor(t_a, QV(2), -1.0, scb3(Q_AKR, gs),
                                       op0=OP.mult, op1=OP.add)
        nc.vector.tensor_mul(t_a, w_w, t_a)
        nc.vector.tensor_add(dots_r, t_a, QV(1))

        ss_r = ctile("ss_r", gi)
        a1 = ctile("alg_a1", gi)
        nc.vector.tensor_sub(a1, QV(3), QV(6))  # sma - sme
        a2 = ctile("alg_a2", gi)
        nc.vector.scalar_tensor_tensor(a2, QV(4), -2.0, scb3(Q_AA, gs),
                                       op0=OP.mult, op1=OP.add)
        nc.vector.tensor_add(a2, a2, QV(7))  # + sme2
        h1 = ctile("alg_h1", gi)
        nc.vector.tensor_mul(h1, w_w, a2)
        nc.vector.scalar_tensor_tensor(h1, a1, 2.0, h1,
                                       op0=OP.mult, op1=OP.add)
        nc.vector.tensor_mul(h1, w_w, h1)
        nc.vector.tensor_add(ss_r, h1, QV(5))  # + ssm

        w_r = work.tile([P, GRP, CH], BF16, tag=f"w_r_g{gi}",
                        name=f"w_r_g{gi}")
        w_chain_all(dots_r, ss_r, pw_r[:, gs:gs + GRP],
                    8, gs, gi, w_r)

        # wrv4[p, bb, c, 0] = w_r ; [.., 1] = w_r*w_w  (bf16 for rv moving)
        wrv4 = work.tile([P, GRP, CH, 2], BF16, tag=f"wrv_g{gi}",
                         name=f"wrv_g{gi}")
        wrv4s[gi] = wrv4
        nc.vector.tensor_copy(wrv4[:, :, :, 0], w_r)
        wrw = ctile("wrw", gi)
        nc.vector.tensor_mul(wrw, w_r, w_w)
        nc.vector.tensor_copy(wrv4[:, :, :, 1], wrw)
        # swr[b] = sum_n w_r*w_w
        swc = gtile("swc", gi, F32)
        nc.vector.reduce_sum(swc, wrw, axis=mybir.AxisListType.X)
        ps_sw = ps_misc.tile([128, 144], F32, tag="pm")
        nc.tensor.matmul(ps_sw[0:GRP, 0:1], swc, ones_sb[:, 0:1],
                         start=True, stop=True)
        swr_c = work.tile([GRP, 1], F32, tag=f"swr_c_g{gi}",
                          name=f"swr_c_g{gi}")
        nc.vector.tensor_copy(swr_c, ps_sw[0:GRP, 0:1])
        ps_swt = ps_misc.tile([128, 144], F32, tag="pm")
        nc.tensor.transpose(ps_swt[0:1, 0:GRP], swr_c,
                            ident_sb[0:GRP, 0:GRP])
        gs2 = gi * GRP
        nc.vector.tensor_copy(swr_sb[:, gs2:gs2 + GRP], ps_swt[0:1, 0:GRP])

    def emit_rv(b):
        # rv contraction: memory chunk-pair [128, 128] stationary (FWL),
        # [w_r, w_r*w_w] for both chunks as 4-wide moving; chunk halves
        # land on partition halves of a [128, 4] accumulating PSUM.
        gi, bb = b // GRP, b % GRP
        wrv4 = wrv4s[gi]
        ps_rv = ps_rvp.tile([128, 4], F32, tag="ps_rv")
        for q in range(NPAIR):
            lhs = m16s[b][:, 2 * q:2 * q + 2, :].rearrange("p c d -> p (c d)")
            rhs = wrv4[:, bb, 2 * q:2 * q + 2, :].rearrange("p c j -> p (c j)")
            nc.tensor.matmul(ps_rv, lhs, rhs,
                             start=(q == 0), stop=(q == NPAIR - 1))
        # valid: partitions 0-63 <- cols 0:2 (chunk-even), 64-127 <- 2:4
        nc.vector.tensor_copy(r12_all[0:64, b, :], ps_rv[0:64, 0:2])
        nc.vector.tensor_copy(r12_all[64:128, b, :], ps_rv[64:128, 2:4])

    # ---------------- emission schedule (software pipeline) ----------------
    emit_batch(0)
    emit_batch(1)
    emit_batch(2)
    emit_batch(3)
    emit_chain_write(0)
    emit_batch(4)
    emit_chain_read(0)
    emit_batch(5)
    emit_rv(0)
    emit_rv(1)
    emit_batch(6)
    emit_rv(2)
    emit_rv(3)
    emit_batch(7)
    emit_chain_write(1)
    emit_chain_read(1)
    for b in range(4, 8):
        emit_rv(b)

    # ---------------- read-vector assembly (all batches) ----------------
    # rvt_full[(c',d), b] = r1 - e*r2 + a*swr  on both partition halves,
    # then one transpose and a free-axis fold of the halves.
    ps_swb = ps_misc.tile([128, 144], F32, tag="pm")
    nc.tensor.matmul(ps_swb[:, 0:BL], ones_sb[0:1, :], swr_sb,
                     start=True, stop=True)
    rvt = work.tile([128, BL], F32, tag="rvt", name="rvt")
    nc.vector.tensor_mul(rvt, e2_sb, r12_all[:, :, 1])   # e * r2
    nc.vector.tensor_sub(rvt, r12_all[:, :, 0], rvt)     # r1 - e*r2
    m3 = work.tile([128, BL], F32, tag="m3", name="m3")
    nc.vector.tensor_mul(m3, a2_sb, ps_swb[:, 0:BL])     # a * sum(wr*ww)
    nc.vector.tensor_add(rvt, rvt, m3)
    ps_rvo = ps_misc.tile([128, 144], F32, tag="pm")
    nc.tensor.transpose(ps_rvo[0:BL, 0:128], rvt, ident_sb)
    rvo_sb = work.tile([BL, 128], F32, tag="rvo_sb", name="rvo_sb")
    nc.vector.tensor_copy(rvo_sb, ps_rvo[0:BL, 0:128])
    nc.vector.tensor_add(out_sb[:, C:C + D], rvo_sb[:, 0:64],
                         rvo_sb[:, 64:128])

    nc.sync.dma_start(out=out_d[:], in_=out_sb)
    ctx.close()


# ---------------------------------------------------------------------------
# host-side driver
# ---------------------------------------------------------------------------
_NC = None


def _get_module():
    global _NC
    if _NC is None:
        _NC = _build_module()
    return _NC


def _consts():
    ident = np.eye(128, dtype=np.float32)
    onest = np.ones((128, 128), np.float32)
    permu = np.zeros((128, 128), np.float32)
    permd = np.zeros((128, 128), np.float32)
    for m in range(128):
        permu[(m + 1) % 128, m] = 1.0
        permd[(m - 1) % 128, m] = 1.0
    sel = np.zeros((32, NQ * 128), np.float32)
    for q in range(NQ):
        sel[q, q * 128:(q + 1) * 128] = 1.0
    return ident, onest, permu, permd, sel


def kernel(**inputs):
    from concourse.bass_utils import run_bass_kernel_spmd

    nc = _get_module()
    f = lambda k: np.ascontiguousarray(np.asarray(inputs[k], np.float32))

    whead = np.concatenate([
        f("Wk_r"), f("Wb_r"), f("Wg_r"), f("Ws_r"), f("Wgam_r"),
        f("Wk_w"), f("Wb_w"), f("Wg_w"), f("Ws_w"), f("Wgam_w"),
        f("We_w"), f("Wa_w")], axis=1)
    bhead = np.concatenate([
        f("bk_r"), f("bb_r"), f("bg_r"), f("bs_r"), f("bgam_r"),
        f("bk_w"), f("bb_w"), f("bg_w"), f("bs_w"), f("bgam_w"),
        f("be_w"), f("ba_w")])
    ident, onest, permu, permd, sel = _consts()

    mem = f("prev_memory")
    x = f("x")
    rv = f("prev_read_vector")
    prw = f("prev_read_weights")
    pww = f("prev_write_weights")
    shared = dict(wctrl=f("W_ctrl"), bctrl=f("b_ctrl"), whead=whead,
                  bhead=bhead, ident=ident, onest=onest, permu=permu,
                  permd=permd, sel=sel)
    in_maps = []
    for c in range(NCORES):
        sl = slice(c * BL, (c + 1) * BL)
        in_maps.append(dict(
            mem=np.ascontiguousarray(mem[sl]),
            x=np.ascontiguousarray(x[sl]),
            rv=np.ascontiguousarray(rv[sl]),
            prw=np.ascontiguousarray(prw[sl]),
            pww=np.ascontiguousarray(pww[sl]),
            **shared))
    res = run_bass_kernel_spmd(nc, in_maps, list(range(NCORES)))
    return np.concatenate([res.results[c]["out"] for c in range(NCORES)],
                          axis=0).astype(np.float32)
